# revision 1
# baseline (speedup 1.0000x reference)
"""TRN2 Bass kernel for nn_Attention_23493471109551.

Full attention layer: QKV projections + interleaved RoPE + causal softmax
attention + output projection, for B=4, S=2048, D=1024, H=16, Dh=64, fp32.

Sharding: 8 cores = 4 batches x 2 head-groups (8 heads each).  Each core
computes its batch/head-group's attention and a partial output projection
(W_o row-block); host sums the two partials per batch.

All matmuls run in fp32r (fp32 rounded to 11-bit mantissa, 1 cyc/row on the
PE at N>=512 vs 4 for fp32).  Inputs are pre-rounded on host; on-device
intermediates are rounded by the producing engine writing float32r tiles.

Layout strategy (per core):
  qpT/kpT: [dh-on-partitions, S]  (heads stacked 2-per-128-partitions)
  scores computed TRANSPOSED [sk, sq] so probs feed the PV matmul directly
  (no per-tile transposes); softmax denominator comes free as a ones-column
  appended to V (fused into the PV matmul, normalized once per [65,512]
  output block); causal mask is an additive -1e30 on the score PSUM applied
  only to the diagonal 128x128 blocks, with score/exp/PV column ranges
  trimmed to the causal triangle; RoPE pair-mixing uses a DVE stream-shuffle.

Measured (8 cores, NTFF profile): ~800-880 us per core, rel err ~1.8e-4.
"""
import math
import numpy as np

import concourse.bass as bass
import concourse.tile as tile
import concourse.mybir as mybir
from concourse import bacc, bass_utils

# problem constants
B, S, D = 4, 2048, 1024
H, Dh = 16, 64
EQ, EV = 2048, 1024          # q/k and v input feature dims
F = 512                      # features per core (8 heads x 64)
P = 128
N_CORES = 8
SCALE = 1.0 / math.sqrt(D)   # 1/32
ROPE_BASE = 10000.0
NEG = -1.0e30
SWAP_MASK = [i ^ 1 for i in range(32)]

F32 = mybir.dt.float32
F32R = mybir.dt.float32r

# test hooks (harness ignores these)
KERNEL_TRACE = False
LAST_RESULT = None

_nc_cache = None


def _round_fp32r(x: np.ndarray) -> np.ndarray:
    """Round fp32 array to the fp32r grid (11-bit mantissa, RNE)."""
    u = np.ascontiguousarray(x, dtype=np.float32).view(np.uint32)
    low = u & np.uint32(0xFFF)
    hi = u >> np.uint32(12)
    round_up = (low > np.uint32(0x800)) | (
        (low == np.uint32(0x800)) & ((hi & np.uint32(1)) == 1)
    )
    hi = hi + round_up.astype(np.uint32)
    return (hi << np.uint32(12)).view(np.float32)


def _build_nc():
    nc = bacc.Bacc("TRN2", target_bir_lowering=False, debug=False)
    qT = nc.dram_tensor("qT", [EQ, S], F32R, kind="ExternalInput").ap()
    kT = nc.dram_tensor("kT", [EQ, S], F32R, kind="ExternalInput").ap()
    vT = nc.dram_tensor("vT", [EV, S], F32R, kind="ExternalInput").ap()
    wqT = nc.dram_tensor("wqT", [EQ, F], F32R, kind="ExternalInput").ap()
    wkT = nc.dram_tensor("wkT", [EQ, F], F32R, kind="ExternalInput").ap()
    wvT = nc.dram_tensor("wvT", [EV, F], F32R, kind="ExternalInput").ap()
    woT = nc.dram_tensor("woT", [F, D], F32R, kind="ExternalInput").ap()
    cosf = nc.dram_tensor("cosf", [P, S], F32, kind="ExternalInput").ap()
    sinf = nc.dram_tensor("sinf", [P, S], F32, kind="ExternalInput").ap()
    maskA = nc.dram_tensor("maskA", [P, P], F32, kind="ExternalInput").ap()
    out = nc.dram_tensor("out", [S, D], F32, kind="ExternalOutput").ap()

    EXP = mybir.ActivationFunctionType.Exp

    with tile.TileContext(nc) as tc:
        with (
            tc.tile_pool(name="consts", bufs=1) as consts,
            tc.tile_pool(name="persist", bufs=1) as persist,
            tc.tile_pool(name="qt", bufs=5) as qt_pool,
            tc.tile_pool(name="wsmall", bufs=6) as w_pool,
            tc.tile_pool(name="rope", bufs=2) as rope_pool,
            tc.tile_pool(name="expp", bufs=3) as exp_pool,
            tc.tile_pool(name="norm", bufs=2) as norm_pool,
            tc.tile_pool(name="outsb", bufs=3) as out_pool,
            tc.tile_pool(name="attnc", bufs=2) as attnc_pool,
            tc.tile_pool(name="ps1", bufs=4, space="PSUM") as ps1,
            tc.tile_pool(name="ps2", bufs=2, space="PSUM") as ps2,
        ):
            # ---- persistent activations
            qpT = persist.tile([P, 4, S], F32R, tag="qpT")
            kpT = persist.tile([P, 4, S], F32R, tag="kpT")
            vpa = persist.tile([P, 16, 8, 65], F32R, tag="vpa")

            nc.vector.memset(vpa[:].bitcast(F32), 1.0)  # ones col; 0:64 overwritten

            # ---- constants (emitted after the first proj tiles get queue priority)
            cos_t = consts.tile([P, S], F32, tag="cos")
            sin_t = consts.tile([P, S], F32, tag="sin")
            mask_t = consts.tile([P, P], F32, tag="mask")
            wo_t = consts.tile([P, 4, D], F32R, tag="woT")

            # ---- q/k projections + rope (qpT[f, s] = sum_e WT[e,f] * xT[e,s])
            _const_dmas_emitted = False
            for src, wsrc, dstT in ((qT, wqT, qpT), (kT, wkT, kpT)):
                for sc in range(4):
                    ssl = slice(sc * 512, (sc + 1) * 512)
                    psums = [None] * 4
                    for e in range(16):
                        at = qt_pool.tile([P, 512], F32R, tag="qt")
                        nc.sync.dma_start(at[:], src[e * P:(e + 1) * P, ssl])
                        wt = w_pool.tile([P, F], F32R, tag="w")
                        nc.sync.dma_start(wt[:], wsrc[e * P:(e + 1) * P, :])
                        for ci in range(4):
                            if psums[ci] is None:
                                psums[ci] = ps1.tile([P, 512], F32, tag="b1",
                                                     name=f"psq{ci}")
                            nc.tensor.matmul(psums[ci][:], wt[:, ci * P:(ci + 1) * P],
                                             at[:], start=(e == 0), stop=(e == 15))
                    if not _const_dmas_emitted:
                        _const_dmas_emitted = True
                        nc.gpsimd.dma_start(cos_t[:], cosf)
                        nc.gpsimd.dma_start(sin_t[:], sinf)
                        nc.gpsimd.dma_start(mask_t[:], maskA)
                        for _ci in range(4):
                            nc.gpsimd.dma_start(wo_t[:, _ci, :],
                                                woT[_ci * P:(_ci + 1) * P, :])
                    # rope: out = x*cos + pairswap(x*sin')
                    for ci in range(4):
                        a_t = rope_pool.tile([P, 512], F32, tag="ropeA")
                        nc.vector.tensor_mul(a_t[:], psums[ci][:], cos_t[:, ssl])
                        c_t = rope_pool.tile([P, 512], F32, tag="ropeC")
                        nc.vector.tensor_mul(c_t[:], psums[ci][:], sin_t[:, ssl])
                        c2_t = rope_pool.tile([P, 512], F32, tag="ropeC")
                        nc.vector.stream_shuffle(c2_t[:], c_t[:], SWAP_MASK)
                        nc.vector.tensor_add(dstT[:, ci, ssl], a_t[:], c2_t[:])

            # ---- v projection (vp[s, f] = sum_e vT[e,s] * wvT[e,f])
            for stq in range(4):
                psv = [ps1.tile([P, 512], F32, tag="b1", name=f"psv{i}") for i in range(4)]
                for e in range(8):
                    wvt = w_pool.tile([P, F], F32R, tag="w")
                    nc.sync.dma_start(wvt[:], wvT[e * P:(e + 1) * P, :])
                    vt = w_pool.tile([P, F], F32R, tag="w", name="vtrow")
                    nc.sync.dma_start(
                        vt[:], vT[e * P:(e + 1) * P, stq * 512:(stq + 1) * 512])
                    for j in range(4):
                        nc.tensor.matmul(psv[j][:], vt[:, j * P:(j + 1) * P], wvt[:],
                                         start=(e == 0), stop=(e == 7))
                for j in range(4):
                    st = stq * 4 + j
                    nc.scalar.copy(vpa[:, st, :, 0:64],
                                   psv[j][:].rearrange("p (h d) -> p h d", h=8))

            # ---- attention (scoresT layout) + output projection, per s-chunk
            for c in range(4):
                ssl = slice(c * 512, (c + 1) * 512)
                nt = 4 * (c + 1)
                attn_c = attnc_pool.tile([P, 4, 512], F32R, tag="attn_c")
                for hp in range(4):
                    po_a = ps1.tile([P, 512], F32, tag="b1")
                    po_b = ps1.tile([P, 512], F32, tag="b1")
                    for t in range(nt):
                        tsl = slice(t * P, (t + 1) * P)
                        rr = 128 * (t - 4 * c) if t >= 4 * c else 0
                        qsl = slice(c * 512 + rr, (c + 1) * 512)
                        ps_s = ps2.tile([P, 1024], F32, tag="b2")
                        nc.tensor.matmul(ps_s[:, rr:512], kpT[0:64, hp, tsl],
                                         qpT[0:64, hp, qsl], start=True, stop=True)
                        nc.tensor.matmul(ps_s[:, 512 + rr:1024], kpT[64:128, hp, tsl],
                                         qpT[64:128, hp, qsl], start=True, stop=True)
                        sv = ps_s[:].rearrange("p (two n) -> p two n", two=2)
                        if t >= 4 * c:
                            nc.vector.tensor_add(
                                sv[:, :, rr:rr + 128], sv[:, :, rr:rr + 128],
                                mask_t[:, None, :].to_broadcast((P, 2, P)))
                        et = exp_pool.tile([P, 2, 512], F32R, tag="exp")
                        nc.scalar.activation(et[:, :, rr:512], sv[:, :, rr:512],
                                             EXP, scale=SCALE)
                        nc.tensor.matmul(po_a[0:65, rr:512], vpa[:, t, 2 * hp, :],
                                         et[:, 0, rr:512], start=(t == 0), stop=(t == nt - 1))
                        nc.tensor.matmul(po_b[0:65, rr:512], vpa[:, t, 2 * hp + 1, :],
                                         et[:, 1, rr:512], start=(t == 0), stop=(t == nt - 1))
                    # normalize by the ones-column sum (row 64)
                    for half, po in ((0, po_a), (1, po_b)):
                        posb = norm_pool.tile([65, 512], F32, tag="posb")
                        nc.scalar.copy(posb[:], po[0:65, :])
                        rc = norm_pool.tile([1, 512], F32, tag="recip")
                        nc.vector.reciprocal(rc[:], posb[64:65, :])
                        bc = norm_pool.tile([64, 512], F32, tag="bcast")
                        nc.gpsimd.partition_broadcast(bc[:], rc[:])
                        nc.vector.tensor_mul(
                            attn_c[64 * half:64 * (half + 1), hp, :],
                            posb[0:64, :], bc[:])
                # W_o for this chunk's 4 s-tiles
                for j in range(4):
                    pw = [ps1.tile([P, 512], F32, tag="b1", name=f"pw{i}") for i in range(2)]
                    for ci in range(4):
                        for oc in range(2):
                            nc.tensor.matmul(pw[oc][:], attn_c[:, ci, j * P:(j + 1) * P],
                                             wo_t[:, ci, oc * 512:(oc + 1) * 512],
                                             start=(ci == 0), stop=(ci == 3))
                    row = (4 * c + j) * P
                    for oc in range(2):
                        ot = out_pool.tile([P, 512], F32, tag="osb")
                        nc.vector.tensor_copy(ot[:], pw[oc][:])
                        nc.sync.dma_start(out[row:row + P, oc * 512:(oc + 1) * 512],
                                          ot[:])
    nc.compile()
    return nc


def _tables():
    inv = (1.0 / (ROPE_BASE ** (np.arange(0, Dh, 2, dtype=np.float32) / Dh))
           ).astype(np.float32)                      # [32]
    pos = np.arange(S, dtype=np.float32)
    ang = pos[:, None] * inv[None, :]                # [S, 32]
    cos = np.cos(ang).astype(np.float32)
    sin = np.sin(ang).astype(np.float32)
    d = np.arange(P) % Dh
    i = d // 2
    cosf = np.ascontiguousarray(cos[:, i].T)         # [128, S]
    sgn = np.where(d % 2 == 0, 1.0, -1.0).astype(np.float32)
    sinf = np.ascontiguousarray(sin[:, i].T * sgn[:, None]).astype(np.float32)

    p = np.arange(P)
    j = np.arange(P)
    maskA = np.where(p[:, None] <= j[None, :], 0.0, NEG).astype(np.float32)
    return cosf, sinf, maskA


def kernel(q, k, v, W_q, W_k, W_v, W_o):
    global _nc_cache, LAST_RESULT
    if _nc_cache is None:
        _nc_cache = _build_nc()
    nc = _nc_cache

    cosf, sinf, maskA = _tables()
    q = np.asarray(q, dtype=np.float32)
    k = np.asarray(k, dtype=np.float32)
    v = np.asarray(v, dtype=np.float32)
    W_q = np.asarray(W_q, dtype=np.float32)
    W_k = np.asarray(W_k, dtype=np.float32)
    W_v = np.asarray(W_v, dtype=np.float32)
    W_o = np.asarray(W_o, dtype=np.float32)

    in_maps = []
    for b in range(B):
        qTb = _round_fp32r(q[b].T)
        kTb = _round_fp32r(k[b].T)
        vTb = _round_fp32r(v[b].T)
        for g in range(2):
            fs = slice(g * F, (g + 1) * F)
            in_maps.append({
                "qT": qTb, "kT": kTb, "vT": vTb,
                "wqT": _round_fp32r(W_q[fs, :].T),
                "wkT": _round_fp32r(W_k[fs, :].T),
                "wvT": _round_fp32r(W_v[fs, :].T),
                "woT": _round_fp32r(W_o[:, fs].T),
                "cosf": cosf, "sinf": sinf, "maskA": maskA,
            })

    res = bass_utils.run_bass_kernel_spmd(
        nc, in_maps, core_ids=list(range(N_CORES)), trace=KERNEL_TRACE)
    LAST_RESULT = res

    final = np.empty((B, S, D), dtype=np.float32)
    for b in range(B):
        final[b] = res.results[2 * b]["out"] + res.results[2 * b + 1]["out"]
    return final



# revision 5
# speedup vs baseline: 1.4957x; 1.4957x over previous
"""TRN2 Bass kernel for nn_Attention_23493471109551 — v2.

Full attention layer: QKV projections + interleaved RoPE + causal softmax
attention + output projection, for B=4, S=2048, D=1024, H=16, Dh=64.

Sharding: 8 cores = 4 batches x 2 head-groups (8 heads each).  Each core
computes its batch/head-group's attention and a partial output projection
(W_o row-block); host sums the two partials per batch.

v2 vs v1:
  * bf16 everywhere on the matmul paths (1 cyc/row at ANY moving size;
    kills the fp32r <256-col 4x penalty), fp32 PSUM accumulation.
  * all weights persist in SBUF (loaded once; v1 re-read 33MB from HBM).
  * chunk-pipelined schedule: projection matmuls of chunk c+1 and the
    W_o matmuls of chunk c-1 are interleaved into attention(c)'s
    score->exp->PV loop as PE filler so the PE never idles long enough
    for the HAM clock-gate to re-throttle, and the ACT-bound exp stream
    overlaps PE work instead of serializing with it.
  * software-pipelined attention inner loop (score(t+1) issued before
    PV(t)) so PV's wait on exp(t) is hidden.
  * reciprocal_approx_fast for softmax denominators (v1 used full-precision
    DVE reciprocal: 3.3us per call, 106us total).
"""
import math
from collections import deque

import numpy as np
import ml_dtypes

import concourse.bass as bass
import concourse.tile as tile
import concourse.mybir as mybir
from concourse import bacc, bass_utils

# problem constants
B, S, D = 4, 2048, 1024
H, Dh = 16, 64
EQ, EV = 2048, 1024          # q/k and v input feature dims
F = 512                      # features per core (8 heads x 64)
P = 128
N_CORES = 8
SCALE = 1.0 / math.sqrt(D)   # 1/32
ROPE_BASE = 10000.0
NEG = -1.0e30
SWAP_MASK = [i ^ 1 for i in range(32)]

F32 = mybir.dt.float32
BF16 = mybir.dt.bfloat16
NPBF = ml_dtypes.bfloat16

# test hooks (harness ignores these)
KERNEL_TRACE = False
LAST_RESULT = None

_nc_cache = None


def _build_nc():
    nc = bacc.Bacc("TRN2", target_bir_lowering=False, debug=False)
    qT = nc.dram_tensor("qT", [EQ, S], BF16, kind="ExternalInput").ap()
    kT = nc.dram_tensor("kT", [EQ, S], BF16, kind="ExternalInput").ap()
    vT = nc.dram_tensor("vT", [EV, S], BF16, kind="ExternalInput").ap()
    wqT = nc.dram_tensor("wqT", [EQ, F], BF16, kind="ExternalInput").ap()
    wkT = nc.dram_tensor("wkT", [EQ, F], BF16, kind="ExternalInput").ap()
    wvT = nc.dram_tensor("wvT", [EV, F], BF16, kind="ExternalInput").ap()
    woT = nc.dram_tensor("woT", [F, D], BF16, kind="ExternalInput").ap()
    cosf = nc.dram_tensor("cosf", [P, S], F32, kind="ExternalInput").ap()
    sinf = nc.dram_tensor("sinf", [P, S], F32, kind="ExternalInput").ap()
    maskA = nc.dram_tensor("maskA", [P, P], F32, kind="ExternalInput").ap()
    out = nc.dram_tensor("out", [S, D], F32, kind="ExternalOutput").ap()

    EXP = mybir.ActivationFunctionType.Exp

    with tile.TileContext(nc) as tc:
        with (
            tc.tile_pool(name="consts", bufs=1) as consts,
            tc.tile_pool(name="persist", bufs=1) as persist,
            tc.tile_pool(name="qin", bufs=3) as qin_pool,
            tc.tile_pool(name="vin", bufs=2) as vin_pool,
            tc.tile_pool(name="rope", bufs=2) as rope_pool,
            tc.tile_pool(name="expp", bufs=3) as et_pool,
            tc.tile_pool(name="norm", bufs=2) as norm_pool,
            tc.tile_pool(name="attnc", bufs=2) as attnc_pool,
            tc.tile_pool(name="outsb", bufs=2) as out_pool,
            tc.tile_pool(name="psA", bufs=2, space="PSUM") as psA,
            tc.tile_pool(name="psB", bufs=1, space="PSUM") as psB,
        ):
            # ---- persistent SBUF
            wq_sb = consts.tile([P, 16, F], BF16, tag="wq")
            wk_sb = consts.tile([P, 16, F], BF16, tag="wk")
            wv_sb = consts.tile([P, 8, F], BF16, tag="wv")
            wo_sb = consts.tile([P, 4, D], BF16, tag="wo")
            cos_t = consts.tile([P, S], F32, tag="cos")
            sin_t = consts.tile([P, S], F32, tag="sin")
            mask_t = consts.tile([P, P], F32, tag="mask")

            qpT = persist.tile([P, 4, S], BF16, tag="qpT")
            kpT = persist.tile([P, 4, S], BF16, tag="kpT")
            vpa = persist.tile([P, 16, 8, 65], BF16, tag="vpa")

            nc.vector.memset(vpa[:], 1.0)  # ones col; 0:64 overwritten

            # ---- input DMA helpers -------------------------------------
            def dma_qk_inputs(c):
                """Start DMAs for chunk c's q/k e-tiles (2 halves each)."""
                ssl = slice(c * 512, (c + 1) * 512)
                tiles = {}
                for key, src, eng in (("q", qT, nc.sync), ("k", kT, nc.scalar)):
                    hs = []
                    for h in range(2):
                        t = qin_pool.tile([P, 8, 512], BF16, tag="qin",
                                          name=f"{key}in{c}h{h}")
                        eng.dma_start(
                            t[:],
                            src[h * 1024:(h + 1) * 1024, ssl]
                            .rearrange("(i p) s -> p i s", p=P))
                        hs.append(t)
                    tiles[key] = hs
                return tiles

            def dma_v_input(c):
                ssl = slice(c * 512, (c + 1) * 512)
                t = vin_pool.tile([P, 8, 512], BF16, tag="vin", name=f"vin{c}")
                nc.gpsimd.dma_start(
                    t[:], vT[:, ssl].rearrange("(i p) s -> p i s", p=P))
                return t

            # ---- projection task generators ----------------------------
            def qk_mm_task(w_sb, tiles, cp, half, pstate):
                if half == 0:
                    pstate["ps"] = psA.tile([P, 2, 512], F32, tag="pq", bufs=1,
                                            name="psq")
                ps = pstate["ps"]
                for e in range(half * 8, half * 8 + 8):
                    stile = tiles[e // 8]
                    for i in range(2):
                        ci = 2 * cp + i
                        nc.tensor.matmul(
                            ps[:, i, :],
                            w_sb[:, e, ci * P:(ci + 1) * P],
                            stile[:, e % 8, :],
                            start=(e == 0), stop=(e == 15))

            def rope_task(dstT, cp, pstate, ssl):
                ps = pstate["ps"]
                a = rope_pool.tile([P, 2, 512], BF16, tag="ra")
                nc.vector.tensor_mul(
                    a[:], ps[:], cos_t[:, None, ssl].to_broadcast((P, 2, 512)))
                cc_ = rope_pool.tile([P, 2, 512], BF16, tag="rb")
                nc.vector.tensor_mul(
                    cc_[:], ps[:], sin_t[:, None, ssl].to_broadcast((P, 2, 512)))
                c2 = rope_pool.tile([P, 2, 512], BF16, tag="rc2")
                nc.vector.stream_shuffle(c2[:], cc_[:], SWAP_MASK)
                nc.vector.tensor_add(dstT[:, 2 * cp:2 * cp + 2, ssl], a[:], c2[:])

            def v_mm_task(vt, jp, half, pstate):
                if half == 0:
                    pstate["ps"] = psA.tile([P, 2, 512], F32, tag="pq", bufs=1,
                                            name="psv")
                ps = pstate["ps"]
                for e in range(half * 4, half * 4 + 4):
                    for i in range(2):
                        j = 2 * jp + i
                        nc.tensor.matmul(
                            ps[:, i, :],
                            vt[:, e, j * P:(j + 1) * P],
                            wv_sb[:, e, :],
                            start=(e == 0), stop=(e == 7))

            def v_copy_task(c, jp, pstate):
                ps = pstate["ps"]
                for i in range(2):
                    st = c * 4 + 2 * jp + i
                    nc.vector.tensor_copy(
                        vpa[:, st, :, 0:64],
                        ps[:, i, :].rearrange("p (h d) -> p h d", h=8))

            def proj_tasks(c, tiles, vt):
                """Ordered list of emission closures projecting chunk c."""
                ssl = slice(c * 512, (c + 1) * 512)
                tasks = []
                for (w_sb, dstT, key) in ((wq_sb, qpT, "q"), (wk_sb, kpT, "k")):
                    for cp in range(2):
                        st = {}
                        tasks.append(
                            lambda w=w_sb, t=tiles[key], cp=cp, s=st:
                                qk_mm_task(w, t, cp, 0, s))
                        tasks.append(
                            lambda w=w_sb, t=tiles[key], cp=cp, s=st:
                                qk_mm_task(w, t, cp, 1, s))
                        tasks.append(
                            lambda d=dstT, cp=cp, s=st: rope_task(d, cp, s, ssl))
                for jp in range(2):
                    st = {}
                    tasks.append(lambda v=vt, jp=jp, s=st: v_mm_task(v, jp, 0, s))
                    tasks.append(lambda v=vt, jp=jp, s=st: v_mm_task(v, jp, 1, s))
                    tasks.append(lambda jp=jp, s=st: v_copy_task(c, jp, s))
                return tasks

            # ---- W_o task ----------------------------------------------
            attn_tiles = {}

            def wo_task(cc, j):
                at = attn_tiles[cc]
                pw = psB.tile([P, 2, 512], F32, tag="po", name=f"pw{cc}_{j}")
                for ci in range(4):
                    for oc in range(2):
                        nc.tensor.matmul(
                            pw[:, oc, :],
                            at[:, ci, j * P:(j + 1) * P],
                            wo_sb[:, ci, oc * 512:(oc + 1) * 512],
                            start=(ci == 0), stop=(ci == 3))
                ot = out_pool.tile([P, 2, 512], F32, tag="osb")
                nc.vector.tensor_copy(ot[:], pw[:])
                row = (4 * cc + j) * P
                nc.gpsimd.dma_start(
                    out[row:row + P, :], ot[:].rearrange("p a b -> p (a b)"))

            # ---- attention chunk ---------------------------------------
            def attention_chunk(c, inner, boundary):
                """inner: deque of filler closures run between t-iterations.
                boundary: deque run at hp-group boundaries (need psB free)."""
                nt = 4 * (c + 1)
                attn_c = attnc_pool.tile([P, 4, 512], BF16, tag="attn_c",
                                         name=f"attn{c}")
                attn_tiles[c] = attn_c
                n_iters = 4 * nt
                n_inner = len(inner)
                done = 0
                it = 0
                for hp in range(4):
                    po = psB.tile([P, 2, 512], F32, tag="po", name=f"po{c}_{hp}")
                    prev = None
                    for t in range(nt):
                        tsl = slice(t * P, (t + 1) * P)
                        rr = 128 * (t - 4 * c) if t >= 4 * c else 0
                        qsl = slice(c * 512 + rr, (c + 1) * 512)
                        ps_s = psA.tile([P, 2, 512], F32, tag="b2")
                        nc.tensor.matmul(ps_s[:, 0, rr:512],
                                         kpT[0:64, hp, tsl], qpT[0:64, hp, qsl],
                                         start=True, stop=True)
                        nc.tensor.matmul(ps_s[:, 1, rr:512],
                                         kpT[64:128, hp, tsl],
                                         qpT[64:128, hp, qsl],
                                         start=True, stop=True)
                        if t >= 4 * c:
                            nc.vector.tensor_add(
                                ps_s[:, :, rr:rr + 128], ps_s[:, :, rr:rr + 128],
                                mask_t[:, None, :].to_broadcast((P, 2, P)))
                        et = et_pool.tile([P, 2, 512], BF16, tag="exp")
                        nc.scalar.activation(et[:, :, rr:512], ps_s[:, :, rr:512],
                                             EXP, scale=SCALE)
                        if prev is not None:
                            emit_pv(c, hp, po, prev, nt)
                        # proportional filler pacing
                        it += 1
                        want = (it * n_inner) // n_iters
                        while done < want and inner:
                            inner.popleft()()
                            done += 1
                        prev = (t, et, rr)
                    emit_pv(c, hp, po, prev, nt)
                    # normalize hp: denominators are po[64]
                    den = norm_pool.tile([1, 2, 512], F32, tag="den", bufs=1)
                    nc.vector.tensor_copy(den[:], po[64:65, :, :])
                    rc = norm_pool.tile([1, 2, 512], F32, tag="rc", bufs=1)
                    nc.vector.reciprocal_approx_fast(rc[:], den[:])
                    bc = norm_pool.tile([64, 2, 512], F32, tag="bc", bufs=1)
                    nc.gpsimd.partition_broadcast(bc[:], rc[:])
                    nc.vector.tensor_mul(attn_c[0:64, hp, :],
                                         po[0:64, 0, :], bc[:, 0, :])
                    nc.vector.tensor_mul(attn_c[64:128, hp, :],
                                         po[0:64, 1, :], bc[:, 1, :])
                    if boundary:
                        boundary.popleft()()
                while inner:
                    inner.popleft()()
                while boundary:
                    boundary.popleft()()

            def emit_pv(c, hp, po, prev, nt):
                t, et, rr = prev
                nc.tensor.matmul(po[0:65, 0, rr:512], vpa[:, t, 2 * hp, :],
                                 et[:, 0, rr:512],
                                 start=(t == 0), stop=(t == nt - 1))
                nc.tensor.matmul(po[0:65, 1, rr:512], vpa[:, t, 2 * hp + 1, :],
                                 et[:, 1, rr:512],
                                 start=(t == 0), stop=(t == nt - 1))

            # ---- main schedule -----------------------------------------
            # chunk 0 inputs + weights (interleaved so first MMs start early)
            tiles0 = {}
            ssl0 = slice(0, 512)
            tiles0["q"] = []
            for h in range(2):
                t = qin_pool.tile([P, 8, 512], BF16, tag="qin", name=f"qin0h{h}")
                nc.sync.dma_start(
                    t[:], qT[h * 1024:(h + 1) * 1024, ssl0]
                    .rearrange("(i p) s -> p i s", p=P))
                nc.sync.dma_start(
                    wq_sb[:, 8 * h:8 * h + 8, :],
                    wqT[h * 1024:(h + 1) * 1024, :]
                    .rearrange("(i p) f -> p i f", p=P))
                tiles0["q"].append(t)
            tiles0["k"] = []
            for h in range(2):
                t = qin_pool.tile([P, 8, 512], BF16, tag="qin", name=f"kin0h{h}")
                nc.scalar.dma_start(
                    t[:], kT[h * 1024:(h + 1) * 1024, ssl0]
                    .rearrange("(i p) s -> p i s", p=P))
                nc.scalar.dma_start(
                    wk_sb[:, 8 * h:8 * h + 8, :],
                    wkT[h * 1024:(h + 1) * 1024, :]
                    .rearrange("(i p) f -> p i f", p=P))
                tiles0["k"].append(t)
            # consts + v on the gpsimd queue
            nc.gpsimd.dma_start(cos_t[:], cosf)
            nc.gpsimd.dma_start(sin_t[:], sinf)
            nc.gpsimd.dma_start(mask_t[:], maskA)
            vt0 = dma_v_input(0)
            nc.gpsimd.dma_start(
                wv_sb[:], wvT[:].rearrange("(i p) f -> p i f", p=P))
            nc.gpsimd.dma_start(
                wo_sb[:], woT[:].rearrange("(i p) d -> p i d", p=P))

            # chunk 0 projection inline
            for task in proj_tasks(0, tiles0, vt0):
                task()

            for c in range(4):
                inner = deque()
                boundary = deque()
                if c < 4 - 1:
                    nxt = c + 1
                    tiles = dma_qk_inputs(nxt)
                    vt = dma_v_input(nxt)
                    inner.extend(proj_tasks(nxt, tiles, vt))
                if c >= 1:
                    for j in range(4):
                        boundary.append(lambda cc=c - 1, j=j: wo_task(cc, j))
                attention_chunk(c, inner, boundary)

            # final W_o for chunk 3
            for j in range(4):
                wo_task(3, j)

    nc.compile()
    return nc


def _tables():
    inv = (1.0 / (ROPE_BASE ** (np.arange(0, Dh, 2, dtype=np.float32) / Dh))
           ).astype(np.float32)                      # [32]
    pos = np.arange(S, dtype=np.float32)
    ang = pos[:, None] * inv[None, :]                # [S, 32]
    cos = np.cos(ang).astype(np.float32)
    sin = np.sin(ang).astype(np.float32)
    d = np.arange(P) % Dh
    i = d // 2
    cosf = np.ascontiguousarray(cos[:, i].T)         # [128, S]
    sgn = np.where(d % 2 == 0, 1.0, -1.0).astype(np.float32)
    sinf = np.ascontiguousarray(sin[:, i].T * sgn[:, None]).astype(np.float32)

    p = np.arange(P)
    j = np.arange(P)
    maskA = np.where(p[:, None] <= j[None, :], 0.0, NEG).astype(np.float32)
    return cosf, sinf, maskA


def _core_inputs(q, k, v, W_q, W_k, W_v, W_o):
    """Build the 8 per-core input maps (bf16 transposed shards)."""
    cosf, sinf, maskA = _tables()
    in_maps = []
    for b in range(B):
        qTb = np.ascontiguousarray(q[b].T).astype(NPBF)
        kTb = np.ascontiguousarray(k[b].T).astype(NPBF)
        vTb = np.ascontiguousarray(v[b].T).astype(NPBF)
        for g in range(2):
            fs = slice(g * F, (g + 1) * F)
            in_maps.append({
                "qT": qTb, "kT": kTb, "vT": vTb,
                "wqT": np.ascontiguousarray(W_q[fs, :].T).astype(NPBF),
                "wkT": np.ascontiguousarray(W_k[fs, :].T).astype(NPBF),
                "wvT": np.ascontiguousarray(W_v[fs, :].T).astype(NPBF),
                "woT": np.ascontiguousarray(W_o[:, fs].T).astype(NPBF),
                "cosf": cosf, "sinf": sinf, "maskA": maskA,
            })
    return in_maps


def kernel(q, k, v, W_q, W_k, W_v, W_o):
    global _nc_cache, LAST_RESULT
    if _nc_cache is None:
        _nc_cache = _build_nc()
    nc = _nc_cache

    q = np.asarray(q, dtype=np.float32)
    k = np.asarray(k, dtype=np.float32)
    v = np.asarray(v, dtype=np.float32)
    W_q = np.asarray(W_q, dtype=np.float32)
    W_k = np.asarray(W_k, dtype=np.float32)
    W_v = np.asarray(W_v, dtype=np.float32)
    W_o = np.asarray(W_o, dtype=np.float32)

    in_maps = _core_inputs(q, k, v, W_q, W_k, W_v, W_o)

    res = bass_utils.run_bass_kernel_spmd(
        nc, in_maps, core_ids=list(range(N_CORES)), trace=KERNEL_TRACE)
    LAST_RESULT = res

    final = np.empty((B, S, D), dtype=np.float32)
    for b in range(B):
        final[b] = res.results[2 * b]["out"] + res.results[2 * b + 1]["out"]
    return final


# revision 7
# speedup vs baseline: 1.5801x; 1.0564x over previous
"""TRN2 Bass kernel for nn_Attention_23493471109551 — v2.

Full attention layer: QKV projections + interleaved RoPE + causal softmax
attention + output projection, for B=4, S=2048, D=1024, H=16, Dh=64.

Sharding: 8 cores = 4 batches x 2 head-groups (8 heads each).  Each core
computes its batch/head-group's attention and a partial output projection
(W_o row-block); host sums the two partials per batch.

v2 vs v1:
  * bf16 everywhere on the matmul paths (1 cyc/row at ANY moving size;
    kills the fp32r <256-col 4x penalty), fp32 PSUM accumulation.
  * all weights persist in SBUF (loaded once; v1 re-read 33MB from HBM).
  * chunk-pipelined schedule: projection matmuls of chunk c+1 and the
    W_o matmuls of chunk c-1 are interleaved into attention(c)'s
    score->exp->PV loop as PE filler so the PE never idles long enough
    for the HAM clock-gate to re-throttle, and the ACT-bound exp stream
    overlaps PE work instead of serializing with it.
  * software-pipelined attention inner loop (score(t+1) issued before
    PV(t)) so PV's wait on exp(t) is hidden.
  * reciprocal_approx_fast for softmax denominators (v1 used full-precision
    DVE reciprocal: 3.3us per call, 106us total).
"""
import math
from collections import deque

import numpy as np
import ml_dtypes

import concourse.bass as bass
import concourse.tile as tile
import concourse.mybir as mybir
from concourse import bacc, bass_utils

# problem constants
B, S, D = 4, 2048, 1024
H, Dh = 16, 64
EQ, EV = 2048, 1024          # q/k and v input feature dims
F = 512                      # features per core (8 heads x 64)
P = 128
N_CORES = 8
SCALE = 1.0 / math.sqrt(D)   # 1/32
ROPE_BASE = 10000.0
NEG = -1.0e30
SWAP_MASK = [i ^ 1 for i in range(32)]

F32 = mybir.dt.float32
BF16 = mybir.dt.bfloat16
NPBF = ml_dtypes.bfloat16

# test hooks (harness ignores these)
KERNEL_TRACE = False
LAST_RESULT = None

_nc_cache = None


def _build_nc():
    nc = bacc.Bacc("TRN2", target_bir_lowering=False, debug=False)
    qT = nc.dram_tensor("qT", [EQ, S], BF16, kind="ExternalInput").ap()
    kT = nc.dram_tensor("kT", [EQ, S], BF16, kind="ExternalInput").ap()
    vT = nc.dram_tensor("vT", [EV, S], BF16, kind="ExternalInput").ap()
    wqT = nc.dram_tensor("wqT", [EQ, F], BF16, kind="ExternalInput").ap()
    wkT = nc.dram_tensor("wkT", [EQ, F], BF16, kind="ExternalInput").ap()
    wvT = nc.dram_tensor("wvT", [EV, F], BF16, kind="ExternalInput").ap()
    woT = nc.dram_tensor("woT", [F, D], BF16, kind="ExternalInput").ap()
    cosf = nc.dram_tensor("cosf", [P, S], F32, kind="ExternalInput").ap()
    sinf = nc.dram_tensor("sinf", [P, S], F32, kind="ExternalInput").ap()
    maskA = nc.dram_tensor("maskA", [P, P], F32, kind="ExternalInput").ap()
    out = nc.dram_tensor("out", [S, D], F32, kind="ExternalOutput").ap()

    EXP = mybir.ActivationFunctionType.Exp

    with tile.TileContext(nc) as tc:
        with (
            tc.tile_pool(name="consts", bufs=1) as consts,
            tc.tile_pool(name="persist", bufs=1) as persist,
            tc.tile_pool(name="qin", bufs=3) as qin_pool,
            tc.tile_pool(name="vin", bufs=2) as vin_pool,
            tc.tile_pool(name="rope", bufs=2) as rope_pool,
            tc.tile_pool(name="expp", bufs=3) as et_pool,
            tc.tile_pool(name="norm", bufs=2) as norm_pool,
            tc.tile_pool(name="attnc", bufs=2) as attnc_pool,
            tc.tile_pool(name="outsb", bufs=2) as out_pool,
            tc.tile_pool(name="psA", bufs=2, space="PSUM") as psA,
            tc.tile_pool(name="psB", bufs=1, space="PSUM") as psB,
        ):
            # ---- persistent SBUF
            wq_sb = consts.tile([P, 16, F], BF16, tag="wq")
            wk_sb = consts.tile([P, 16, F], BF16, tag="wk")
            wv_sb = consts.tile([P, 8, F], BF16, tag="wv")
            wo_sb = consts.tile([P, 4, D], BF16, tag="wo")
            cos_t = consts.tile([P, S], F32, tag="cos")
            sin_t = consts.tile([P, S], F32, tag="sin")
            mask_t = consts.tile([P, P], F32, tag="mask")

            qpT = persist.tile([P, 4, S], BF16, tag="qpT")
            kpT = persist.tile([P, 4, S], BF16, tag="kpT")
            vpa = persist.tile([P, 16, 8, 65], BF16, tag="vpa")

            nc.vector.memset(vpa[:], 1.0)  # ones col; 0:64 overwritten

            # ---- input DMA helpers -------------------------------------
            def dma_qk_inputs(c):
                """Start DMAs for chunk c's q/k e-tiles (2 halves each)."""
                ssl = slice(c * 512, (c + 1) * 512)
                tiles = {}
                for key, src, eng in (("q", qT, nc.sync), ("k", kT, nc.scalar)):
                    hs = []
                    for h in range(2):
                        t = qin_pool.tile([P, 8, 512], BF16, tag="qin",
                                          name=f"{key}in{c}h{h}")
                        eng.dma_start(
                            t[:],
                            src[h * 1024:(h + 1) * 1024, ssl]
                            .rearrange("(i p) s -> p i s", p=P))
                        hs.append(t)
                    tiles[key] = hs
                return tiles

            def dma_v_input(c):
                ssl = slice(c * 512, (c + 1) * 512)
                t = vin_pool.tile([P, 8, 512], BF16, tag="vin", name=f"vin{c}")
                nc.gpsimd.dma_start(
                    t[:], vT[:, ssl].rearrange("(i p) s -> p i s", p=P))
                return t

            # ---- projection task generators ----------------------------
            def qk_mm_task(w_sb, tiles, cp, half, pstate):
                if half == 0:
                    pstate["ps"] = psA.tile([P, 2, 512], F32, tag="pq", bufs=1,
                                            name="psq")
                ps = pstate["ps"]
                for e in range(half * 8, half * 8 + 8):
                    stile = tiles[e // 8]
                    for i in range(2):
                        ci = 2 * cp + i
                        nc.tensor.matmul(
                            ps[:, i, :],
                            w_sb[:, e, ci * P:(ci + 1) * P],
                            stile[:, e % 8, :],
                            start=(e == 0), stop=(e == 15))

            def rope_task(dstT, cp, pstate, ssl):
                ps = pstate["ps"]
                a = rope_pool.tile([P, 2, 512], BF16, tag="ra")
                nc.vector.tensor_mul(
                    a[:], ps[:], cos_t[:, None, ssl].to_broadcast((P, 2, 512)))
                cc_ = rope_pool.tile([P, 2, 512], BF16, tag="rb")
                nc.vector.tensor_mul(
                    cc_[:], ps[:], sin_t[:, None, ssl].to_broadcast((P, 2, 512)))
                c2 = rope_pool.tile([P, 2, 512], BF16, tag="rc2")
                nc.vector.stream_shuffle(c2[:], cc_[:], SWAP_MASK)
                nc.vector.tensor_add(dstT[:, 2 * cp:2 * cp + 2, ssl], a[:], c2[:])

            def v_mm_task(vt, jp, half, pstate):
                if half == 0:
                    pstate["ps"] = psA.tile([P, 2, 512], F32, tag="pq", bufs=1,
                                            name="psv")
                ps = pstate["ps"]
                for e in range(half * 4, half * 4 + 4):
                    for i in range(2):
                        j = 2 * jp + i
                        nc.tensor.matmul(
                            ps[:, i, :],
                            vt[:, e, j * P:(j + 1) * P],
                            wv_sb[:, e, :],
                            start=(e == 0), stop=(e == 7))

            def v_copy_task(c, jp, pstate):
                ps = pstate["ps"]
                for i in range(2):
                    st = c * 4 + 2 * jp + i
                    nc.vector.tensor_copy(
                        vpa[:, st, :, 0:64],
                        ps[:, i, :].rearrange("p (h d) -> p h d", h=8))

            def proj_tasks(c, tiles, vt):
                """(qk_tasks, v_tasks) emission closures projecting chunk c."""
                ssl = slice(c * 512, (c + 1) * 512)
                tasks = []
                for (w_sb, dstT, key) in ((wq_sb, qpT, "q"), (wk_sb, kpT, "k")):
                    for cp in range(2):
                        st = {}
                        tasks.append(
                            lambda w=w_sb, t=tiles[key], cp=cp, s=st:
                                qk_mm_task(w, t, cp, 0, s))
                        tasks.append(
                            lambda w=w_sb, t=tiles[key], cp=cp, s=st:
                                qk_mm_task(w, t, cp, 1, s))
                        tasks.append(
                            lambda d=dstT, cp=cp, s=st: rope_task(d, cp, s, ssl))
                vtasks = []
                for jp in range(2):
                    st = {}
                    vtasks.append(lambda v=vt, jp=jp, s=st: v_mm_task(v, jp, 0, s))
                    vtasks.append(lambda v=vt, jp=jp, s=st: v_mm_task(v, jp, 1, s))
                    vtasks.append(lambda jp=jp, s=st: v_copy_task(c, jp, s))
                return tasks, vtasks

            # ---- W_o task ----------------------------------------------
            attn_tiles = {}

            def wo_task(cc, j):
                at = attn_tiles[cc]
                pw = psB.tile([P, 2, 512], F32, tag="po", name=f"pw{cc}_{j}")
                for ci in range(4):
                    for oc in range(2):
                        nc.tensor.matmul(
                            pw[:, oc, :],
                            at[:, ci, j * P:(j + 1) * P],
                            wo_sb[:, ci, oc * 512:(oc + 1) * 512],
                            start=(ci == 0), stop=(ci == 3))
                ot = out_pool.tile([P, 2, 512], F32, tag="osb")
                nc.vector.tensor_copy(ot[:], pw[:])
                row = (4 * cc + j) * P
                nc.gpsimd.dma_start(
                    out[row:row + P, :], ot[:].rearrange("p a b -> p (a b)"))

            # ---- attention chunk ---------------------------------------
            def attention_chunk(c, inner, boundary, frontload=False):
                """inner: deque of filler closures run between t-iterations.
                boundary: deque run at hp-group boundaries (need psB free).
                frontload: drain inner at 1/iter from the start (needed when
                fillers write tiles read later in THIS chunk)."""
                nt = 4 * (c + 1)
                attn_c = attnc_pool.tile([P, 4, 512], BF16, tag="attn_c",
                                         name=f"attn{c}")
                attn_tiles[c] = attn_c
                n_iters = 4 * nt
                n_inner = len(inner)
                done = 0
                it = 0
                for hp in range(4):
                    po = psB.tile([P, 2, 512], F32, tag="po", name=f"po{c}_{hp}")
                    prev = None
                    for t in range(nt):
                        tsl = slice(t * P, (t + 1) * P)
                        rr = 128 * (t - 4 * c) if t >= 4 * c else 0
                        qsl = slice(c * 512 + rr, (c + 1) * 512)
                        ps_s = psA.tile([P, 2, 512], F32, tag="b2")
                        nc.tensor.matmul(ps_s[:, 0, rr:512],
                                         kpT[0:64, hp, tsl], qpT[0:64, hp, qsl],
                                         start=True, stop=True)
                        nc.tensor.matmul(ps_s[:, 1, rr:512],
                                         kpT[64:128, hp, tsl],
                                         qpT[64:128, hp, qsl],
                                         start=True, stop=True)
                        if t >= 4 * c:
                            nc.vector.tensor_add(
                                ps_s[:, :, rr:rr + 128], ps_s[:, :, rr:rr + 128],
                                mask_t[:, None, :].to_broadcast((P, 2, P)))
                        et = et_pool.tile([P, 2, 512], BF16, tag="exp")
                        nc.scalar.activation(et[:, :, rr:512], ps_s[:, :, rr:512],
                                             EXP, scale=SCALE)
                        if prev is not None:
                            emit_pv(c, hp, po, prev, nt)
                        # proportional filler pacing
                        it += 1
                        want = it if frontload else (it * n_inner) // n_iters
                        while done < want and inner:
                            inner.popleft()()
                            done += 1
                        prev = (t, et, rr)
                    emit_pv(c, hp, po, prev, nt)
                    # normalize hp: denominators are po[64]
                    den = norm_pool.tile([1, 2, 512], F32, tag="den", bufs=1)
                    nc.vector.tensor_copy(den[:], po[64:65, :, :])
                    rc = norm_pool.tile([1, 2, 512], F32, tag="rc", bufs=1)
                    nc.vector.reciprocal_approx_fast(rc[:], den[:])
                    bc = norm_pool.tile([64, 2, 512], F32, tag="bc", bufs=1)
                    nc.gpsimd.partition_broadcast(bc[:], rc[:])
                    nc.vector.tensor_mul(attn_c[0:64, hp, :],
                                         po[0:64, 0, :], bc[:, 0, :])
                    nc.vector.tensor_mul(attn_c[64:128, hp, :],
                                         po[0:64, 1, :], bc[:, 1, :])
                    if boundary:
                        boundary.popleft()()
                while inner:
                    inner.popleft()()
                while boundary:
                    boundary.popleft()()

            def emit_pv(c, hp, po, prev, nt):
                t, et, rr = prev
                nc.tensor.matmul(po[0:65, 0, rr:512], vpa[:, t, 2 * hp, :],
                                 et[:, 0, rr:512],
                                 start=(t == 0), stop=(t == nt - 1))
                nc.tensor.matmul(po[0:65, 1, rr:512], vpa[:, t, 2 * hp + 1, :],
                                 et[:, 1, rr:512],
                                 start=(t == 0), stop=(t == nt - 1))

            # ---- main schedule -----------------------------------------
            # chunk 0 inputs + weights (interleaved so first MMs start early)
            tiles0 = {}
            ssl0 = slice(0, 512)
            tiles0["q"] = []
            for h in range(2):
                t = qin_pool.tile([P, 8, 512], BF16, tag="qin", name=f"qin0h{h}")
                nc.sync.dma_start(
                    t[:], qT[h * 1024:(h + 1) * 1024, ssl0]
                    .rearrange("(i p) s -> p i s", p=P))
                nc.sync.dma_start(
                    wq_sb[:, 8 * h:8 * h + 8, :],
                    wqT[h * 1024:(h + 1) * 1024, :]
                    .rearrange("(i p) f -> p i f", p=P))
                tiles0["q"].append(t)
            tiles0["k"] = []
            for h in range(2):
                t = qin_pool.tile([P, 8, 512], BF16, tag="qin", name=f"kin0h{h}")
                nc.scalar.dma_start(
                    t[:], kT[h * 1024:(h + 1) * 1024, ssl0]
                    .rearrange("(i p) s -> p i s", p=P))
                nc.scalar.dma_start(
                    wk_sb[:, 8 * h:8 * h + 8, :],
                    wkT[h * 1024:(h + 1) * 1024, :]
                    .rearrange("(i p) f -> p i f", p=P))
                tiles0["k"].append(t)
            # consts + v on the gpsimd queue
            nc.gpsimd.dma_start(cos_t[:], cosf)
            nc.gpsimd.dma_start(sin_t[:], sinf)
            nc.gpsimd.dma_start(mask_t[:], maskA)
            vt0 = dma_v_input(0)
            nc.gpsimd.dma_start(
                wv_sb[:], wvT[:].rearrange("(i p) f -> p i f", p=P))
            nc.gpsimd.dma_start(
                wo_sb[:], woT[:].rearrange("(i p) d -> p i d", p=P))

            # chunk 0 projection inline
            qk0, v0 = proj_tasks(0, tiles0, vt0)
            for task in qk0 + v0:
                task()

            deferred_v = None
            for c in range(4):
                inner = deque()
                boundary = deque()
                if deferred_v is not None:
                    inner.extend(deferred_v)
                    deferred_v = None
                if c < 4 - 1:
                    nxt = c + 1
                    tiles = dma_qk_inputs(nxt)
                    vt = dma_v_input(nxt)
                    qk_t, v_t = proj_tasks(nxt, tiles, vt)
                    inner.extend(qk_t)
                    if nxt == 3:
                        deferred_v = v_t   # run inside attention(3) instead
                    else:
                        inner.extend(v_t)
                if c >= 1:
                    for j in range(4):
                        boundary.append(lambda cc=c - 1, j=j: wo_task(cc, j))
                attention_chunk(c, inner, boundary, frontload=(c == 3))

            # final W_o for chunk 3
            for j in range(4):
                wo_task(3, j)

    nc.compile()
    return nc


def _tables():
    inv = (1.0 / (ROPE_BASE ** (np.arange(0, Dh, 2, dtype=np.float32) / Dh))
           ).astype(np.float32)                      # [32]
    pos = np.arange(S, dtype=np.float32)
    ang = pos[:, None] * inv[None, :]                # [S, 32]
    cos = np.cos(ang).astype(np.float32)
    sin = np.sin(ang).astype(np.float32)
    d = np.arange(P) % Dh
    i = d // 2
    cosf = np.ascontiguousarray(cos[:, i].T)         # [128, S]
    sgn = np.where(d % 2 == 0, 1.0, -1.0).astype(np.float32)
    sinf = np.ascontiguousarray(sin[:, i].T * sgn[:, None]).astype(np.float32)

    p = np.arange(P)
    j = np.arange(P)
    maskA = np.where(p[:, None] <= j[None, :], 0.0, NEG).astype(np.float32)
    return cosf, sinf, maskA


def _core_inputs(q, k, v, W_q, W_k, W_v, W_o):
    """Build the 8 per-core input maps (bf16 transposed shards)."""
    cosf, sinf, maskA = _tables()
    in_maps = []
    for b in range(B):
        qTb = np.ascontiguousarray(q[b].T).astype(NPBF)
        kTb = np.ascontiguousarray(k[b].T).astype(NPBF)
        vTb = np.ascontiguousarray(v[b].T).astype(NPBF)
        for g in range(2):
            fs = slice(g * F, (g + 1) * F)
            in_maps.append({
                "qT": qTb, "kT": kTb, "vT": vTb,
                "wqT": np.ascontiguousarray(W_q[fs, :].T).astype(NPBF),
                "wkT": np.ascontiguousarray(W_k[fs, :].T).astype(NPBF),
                "wvT": np.ascontiguousarray(W_v[fs, :].T).astype(NPBF),
                "woT": np.ascontiguousarray(W_o[:, fs].T).astype(NPBF),
                "cosf": cosf, "sinf": sinf, "maskA": maskA,
            })
    return in_maps


def kernel(q, k, v, W_q, W_k, W_v, W_o):
    global _nc_cache, LAST_RESULT
    if _nc_cache is None:
        _nc_cache = _build_nc()
    nc = _nc_cache

    q = np.asarray(q, dtype=np.float32)
    k = np.asarray(k, dtype=np.float32)
    v = np.asarray(v, dtype=np.float32)
    W_q = np.asarray(W_q, dtype=np.float32)
    W_k = np.asarray(W_k, dtype=np.float32)
    W_v = np.asarray(W_v, dtype=np.float32)
    W_o = np.asarray(W_o, dtype=np.float32)

    in_maps = _core_inputs(q, k, v, W_q, W_k, W_v, W_o)

    res = bass_utils.run_bass_kernel_spmd(
        nc, in_maps, core_ids=list(range(N_CORES)), trace=KERNEL_TRACE)
    LAST_RESULT = res

    final = np.empty((B, S, D), dtype=np.float32)
    for b in range(B):
        final[b] = res.results[2 * b]["out"] + res.results[2 * b + 1]["out"]
    return final


# revision 10
# speedup vs baseline: 1.6762x; 1.0608x over previous
"""TRN2 Bass kernel for nn_Attention_23493471109551 — v2.

Full attention layer: QKV projections + interleaved RoPE + causal softmax
attention + output projection, for B=4, S=2048, D=1024, H=16, Dh=64.

Sharding: 8 cores = 4 batches x 2 head-groups (8 heads each).  Each core
computes its batch/head-group's attention and a partial output projection
(W_o row-block); host sums the two partials per batch.

v2 vs v1:
  * bf16 everywhere on the matmul paths (1 cyc/row at ANY moving size;
    kills the fp32r <256-col 4x penalty), fp32 PSUM accumulation.
  * all weights persist in SBUF (loaded once; v1 re-read 33MB from HBM).
  * chunk-pipelined schedule: projection matmuls of chunk c+1 and the
    W_o matmuls of chunk c-1 are interleaved into attention(c)'s
    score->exp->PV loop as PE filler so the PE never idles long enough
    for the HAM clock-gate to re-throttle, and the ACT-bound exp stream
    overlaps PE work instead of serializing with it.
  * software-pipelined attention inner loop (score(t+1) issued before
    PV(t)) so PV's wait on exp(t) is hidden.
  * reciprocal_approx_fast for softmax denominators (v1 used full-precision
    DVE reciprocal: 3.3us per call, 106us total).
"""
import math
from collections import deque

import numpy as np
import ml_dtypes

import concourse.bass as bass
import concourse.tile as tile
import concourse.mybir as mybir
from concourse import bacc, bass_utils

# problem constants
B, S, D = 4, 2048, 1024
H, Dh = 16, 64
EQ, EV = 2048, 1024          # q/k and v input feature dims
F = 512                      # features per core (8 heads x 64)
P = 128
N_CORES = 8
SCALE = 1.0 / math.sqrt(D)   # 1/32
ROPE_BASE = 10000.0
NEG = -1.0e30
SWAP_MASK = [i ^ 1 for i in range(32)]

F32 = mybir.dt.float32
BF16 = mybir.dt.bfloat16
NPBF = ml_dtypes.bfloat16

# test hooks (harness ignores these)
KERNEL_TRACE = False
LAST_RESULT = None

_nc_cache = None


def _build_nc():
    nc = bacc.Bacc("TRN2", target_bir_lowering=False, debug=False)
    qT = nc.dram_tensor("qT", [EQ, S], BF16, kind="ExternalInput").ap()
    kT = nc.dram_tensor("kT", [EQ, S], BF16, kind="ExternalInput").ap()
    vT = nc.dram_tensor("vT", [EV, S], BF16, kind="ExternalInput").ap()
    wqT = nc.dram_tensor("wqT", [EQ, F], BF16, kind="ExternalInput").ap()
    wkT = nc.dram_tensor("wkT", [EQ, F], BF16, kind="ExternalInput").ap()
    wvT = nc.dram_tensor("wvT", [EV, F], BF16, kind="ExternalInput").ap()
    woT = nc.dram_tensor("woT", [F, D], BF16, kind="ExternalInput").ap()
    cosf = nc.dram_tensor("cosf", [P, S], F32, kind="ExternalInput").ap()
    sinf = nc.dram_tensor("sinf", [P, S], F32, kind="ExternalInput").ap()
    maskA = nc.dram_tensor("maskA", [P, P], F32, kind="ExternalInput").ap()
    out = nc.dram_tensor("out", [S, D], F32, kind="ExternalOutput").ap()

    EXP = mybir.ActivationFunctionType.Exp

    with tile.TileContext(nc) as tc:
        with (
            tc.tile_pool(name="consts", bufs=1) as consts,
            tc.tile_pool(name="persist", bufs=1) as persist,
            tc.tile_pool(name="qin", bufs=3) as qin_pool,
            tc.tile_pool(name="vin", bufs=2) as vin_pool,
            tc.tile_pool(name="rope", bufs=2) as rope_pool,
            tc.tile_pool(name="expp", bufs=3) as et_pool,
            tc.tile_pool(name="norm", bufs=2) as norm_pool,
            tc.tile_pool(name="attnc", bufs=2) as attnc_pool,
            tc.tile_pool(name="outsb", bufs=2) as out_pool,
            tc.tile_pool(name="psA", bufs=2, space="PSUM") as psA,
            tc.tile_pool(name="psB", bufs=1, space="PSUM") as psB,
        ):
            # ---- persistent SBUF
            wq_sb = consts.tile([P, 16, F], BF16, tag="wq")
            wk_sb = consts.tile([P, 16, F], BF16, tag="wk")
            wv_sb = consts.tile([P, 8, F], BF16, tag="wv")
            wo_sb = consts.tile([P, 4, D], BF16, tag="wo")
            cos_t = consts.tile([P, S], F32, tag="cos")
            sin_t = consts.tile([P, S], F32, tag="sin")
            mask_t = consts.tile([P, P], F32, tag="mask")

            qpT = persist.tile([P, 4, S], BF16, tag="qpT")
            kpT = persist.tile([P, 4, S], BF16, tag="kpT")
            vpa = persist.tile([P, 16, 8, 65], BF16, tag="vpa")

            nc.vector.memset(vpa[:], 1.0)  # ones col; 0:64 overwritten

            # PE warm-up: ~4us of dummy matmuls on a zeroed tile so the HAM
            # clock-gate reaches K=8/8 while the first input DMAs stream in.
            warm = consts.tile([P, 512], BF16, tag="warm")
            nc.vector.memset(warm[:], 0.0)
            wps = psA.tile([P, 2, 512], F32, tag="b2", name="warmps")
            for wi in range(20):
                nc.tensor.matmul(wps[:, wi % 2, :], warm[:, 0:P], warm[:],
                                 start=True, stop=True)

            # ---- input DMA helpers -------------------------------------
            def dma_qk_inputs(c):
                """Start DMAs for chunk c's q/k e-tiles (2 halves each)."""
                ssl = slice(c * 512, (c + 1) * 512)
                tiles = {}
                for key, src, eng in (("q", qT, nc.sync), ("k", kT, nc.scalar)):
                    hs = []
                    for h in range(2):
                        t = qin_pool.tile([P, 8, 512], BF16, tag="qin",
                                          name=f"{key}in{c}h{h}")
                        eng.dma_start(
                            t[:],
                            src[h * 1024:(h + 1) * 1024, ssl]
                            .rearrange("(i p) s -> p i s", p=P))
                        hs.append(t)
                    tiles[key] = hs
                return tiles

            def dma_v_input(c):
                ssl = slice(c * 512, (c + 1) * 512)
                t = vin_pool.tile([P, 8, 512], BF16, tag="vin", name=f"vin{c}")
                nc.gpsimd.dma_start(
                    t[:], vT[:, ssl].rearrange("(i p) s -> p i s", p=P))
                return t

            # ---- projection task generators ----------------------------
            def qk_mm_task(w_sb, tiles, cp, half, pstate):
                if half == 0:
                    pstate["ps"] = psA.tile([P, 2, 512], F32, tag="pq", bufs=1,
                                            name="psq")
                ps = pstate["ps"]
                for e in range(half * 8, half * 8 + 8):
                    stile = tiles[e // 8]
                    for i in range(2):
                        ci = 2 * cp + i
                        nc.tensor.matmul(
                            ps[:, i, :],
                            w_sb[:, e, ci * P:(ci + 1) * P],
                            stile[:, e % 8, :],
                            start=(e == 0), stop=(e == 15))

            def rope_task(dstT, cp, pstate, ssl):
                ps = pstate["ps"]
                a = rope_pool.tile([P, 2, 512], BF16, tag="ra")
                nc.vector.tensor_mul(
                    a[:], ps[:], cos_t[:, None, ssl].to_broadcast((P, 2, 512)))
                cc_ = rope_pool.tile([P, 2, 512], BF16, tag="rb")
                nc.vector.tensor_mul(
                    cc_[:], ps[:], sin_t[:, None, ssl].to_broadcast((P, 2, 512)))
                c2 = rope_pool.tile([P, 2, 512], BF16, tag="rc2")
                nc.vector.stream_shuffle(c2[:], cc_[:], SWAP_MASK)
                nc.vector.tensor_add(dstT[:, 2 * cp:2 * cp + 2, ssl], a[:], c2[:])

            def v_mm_task(vt, jp, half, pstate):
                if half == 0:
                    pstate["ps"] = psA.tile([P, 2, 512], F32, tag="pq", bufs=1,
                                            name="psv")
                ps = pstate["ps"]
                for e in range(half * 4, half * 4 + 4):
                    for i in range(2):
                        j = 2 * jp + i
                        nc.tensor.matmul(
                            ps[:, i, :],
                            vt[:, e, j * P:(j + 1) * P],
                            wv_sb[:, e, :],
                            start=(e == 0), stop=(e == 7))

            def v_copy_task(c, jp, pstate):
                ps = pstate["ps"]
                for i in range(2):
                    st = c * 4 + 2 * jp + i
                    nc.vector.tensor_copy(
                        vpa[:, st, :, 0:64],
                        ps[:, i, :].rearrange("p (h d) -> p h d", h=8))

            def proj_tasks(c, tiles, vt):
                """(qk_tasks, v_tasks) emission closures projecting chunk c."""
                ssl = slice(c * 512, (c + 1) * 512)
                tasks = []
                for (w_sb, dstT, key) in ((wq_sb, qpT, "q"), (wk_sb, kpT, "k")):
                    for cp in range(2):
                        st = {}
                        tasks.append(
                            lambda w=w_sb, t=tiles[key], cp=cp, s=st:
                                qk_mm_task(w, t, cp, 0, s))
                        tasks.append(
                            lambda w=w_sb, t=tiles[key], cp=cp, s=st:
                                qk_mm_task(w, t, cp, 1, s))
                        tasks.append(
                            lambda d=dstT, cp=cp, s=st: rope_task(d, cp, s, ssl))
                vtasks = []
                for jp in range(2):
                    st = {}
                    vtasks.append(lambda v=vt, jp=jp, s=st: v_mm_task(v, jp, 0, s))
                    vtasks.append(lambda v=vt, jp=jp, s=st: v_mm_task(v, jp, 1, s))
                    vtasks.append(lambda jp=jp, s=st: v_copy_task(c, jp, s))
                return tasks, vtasks

            # ---- W_o task ----------------------------------------------
            attn_tiles = {}

            def wo_task(cc, j, pool=None, ptag="po"):
                at = attn_tiles[cc]
                pw = (pool or psB).tile([P, 2, 512], F32, tag=ptag,
                                        name=f"pw{cc}_{j}")
                for ci in range(4):
                    for oc in range(2):
                        nc.tensor.matmul(
                            pw[:, oc, :],
                            at[:, ci, j * P:(j + 1) * P],
                            wo_sb[:, ci, oc * 512:(oc + 1) * 512],
                            start=(ci == 0), stop=(ci == 3))
                ot = out_pool.tile([P, 2, 512], F32, tag="osb")
                nc.vector.tensor_copy(ot[:], pw[:])
                row = (4 * cc + j) * P
                nc.gpsimd.dma_start(
                    out[row:row + P, :], ot[:].rearrange("p a b -> p (a b)"))

            # ---- attention chunk ---------------------------------------
            def attention_chunk(c, inner, boundary, frontload=False):
                """inner: deque of filler closures run between t-iterations.
                boundary: deque run at hp-group boundaries (need psB free).
                frontload: drain inner at 1/iter from the start (needed when
                fillers write tiles read later in THIS chunk)."""
                nt = 4 * (c + 1)
                attn_c = attnc_pool.tile([P, 4, 512], BF16, tag="attn_c",
                                         name=f"attn{c}")
                attn_tiles[c] = attn_c
                n_iters = 4 * nt
                n_inner = len(inner)
                done = 0
                it = 0
                for hp in range(4):
                    po = psB.tile([P, 2, 512], F32, tag="po", name=f"po{c}_{hp}")
                    prev = None
                    for t in range(nt):
                        tsl = slice(t * P, (t + 1) * P)
                        rr = 128 * (t - 4 * c) if t >= 4 * c else 0
                        qsl = slice(c * 512 + rr, (c + 1) * 512)
                        ps_s = psA.tile([P, 2, 512], F32, tag="b2")
                        nc.tensor.matmul(ps_s[:, 0, rr:512],
                                         kpT[0:64, hp, tsl], qpT[0:64, hp, qsl],
                                         start=True, stop=True)
                        nc.tensor.matmul(ps_s[:, 1, rr:512],
                                         kpT[64:128, hp, tsl],
                                         qpT[64:128, hp, qsl],
                                         start=True, stop=True)
                        if t >= 4 * c:
                            nc.vector.tensor_add(
                                ps_s[:, :, rr:rr + 128], ps_s[:, :, rr:rr + 128],
                                mask_t[:, None, :].to_broadcast((P, 2, P)))
                        et = et_pool.tile([P, 2, 512], BF16, tag="exp")
                        nc.scalar.activation(et[:, :, rr:512], ps_s[:, :, rr:512],
                                             EXP, scale=SCALE)
                        if prev is not None:
                            emit_pv(c, hp, po, prev, nt)
                        # proportional filler pacing
                        it += 1
                        want = it if frontload else (it * n_inner) // n_iters
                        while done < want and inner:
                            inner.popleft()()
                            done += 1
                        prev = (t, et, rr)
                    emit_pv(c, hp, po, prev, nt)
                    # normalize hp: denominators are po[64]
                    den = norm_pool.tile([1, 2, 512], F32, tag="den", bufs=1)
                    nc.vector.tensor_copy(den[:], po[64:65, :, :])
                    rc = norm_pool.tile([1, 2, 512], F32, tag="rc", bufs=1)
                    nc.vector.reciprocal_approx_fast(rc[:], den[:])
                    bc = norm_pool.tile([64, 2, 512], F32, tag="bc", bufs=1)
                    nc.gpsimd.partition_broadcast(bc[:], rc[:])
                    nc.vector.tensor_mul(attn_c[0:64, hp, :],
                                         po[0:64, 0, :], bc[:, 0, :])
                    nc.vector.tensor_mul(attn_c[64:128, hp, :],
                                         po[0:64, 1, :], bc[:, 1, :])
                    if boundary:
                        boundary.popleft()()
                while inner:
                    inner.popleft()()
                while boundary:
                    boundary.popleft()()

            def emit_pv(c, hp, po, prev, nt):
                t, et, rr = prev
                nc.tensor.matmul(po[0:65, 0, rr:512], vpa[:, t, 2 * hp, :],
                                 et[:, 0, rr:512],
                                 start=(t == 0), stop=(t == nt - 1))
                nc.tensor.matmul(po[0:65, 1, rr:512], vpa[:, t, 2 * hp + 1, :],
                                 et[:, 1, rr:512],
                                 start=(t == 0), stop=(t == nt - 1))

            # ---- main schedule -----------------------------------------
            # chunk 0 inputs + weights (interleaved so first MMs start early)
            tiles0 = {}
            ssl0 = slice(0, 512)
            tiles0["q"] = []
            for h in range(2):
                t = qin_pool.tile([P, 8, 512], BF16, tag="qin", name=f"qin0h{h}")
                nc.sync.dma_start(
                    t[:], qT[h * 1024:(h + 1) * 1024, ssl0]
                    .rearrange("(i p) s -> p i s", p=P))
                nc.sync.dma_start(
                    wq_sb[:, 8 * h:8 * h + 8, :],
                    wqT[h * 1024:(h + 1) * 1024, :]
                    .rearrange("(i p) f -> p i f", p=P))
                tiles0["q"].append(t)
            tiles0["k"] = []
            for h in range(2):
                t = qin_pool.tile([P, 8, 512], BF16, tag="qin", name=f"kin0h{h}")
                nc.scalar.dma_start(
                    t[:], kT[h * 1024:(h + 1) * 1024, ssl0]
                    .rearrange("(i p) s -> p i s", p=P))
                nc.scalar.dma_start(
                    wk_sb[:, 8 * h:8 * h + 8, :],
                    wkT[h * 1024:(h + 1) * 1024, :]
                    .rearrange("(i p) f -> p i f", p=P))
                tiles0["k"].append(t)
            # consts + v on the gpsimd queue
            nc.gpsimd.dma_start(cos_t[:], cosf)
            nc.gpsimd.dma_start(sin_t[:], sinf)
            nc.gpsimd.dma_start(mask_t[:], maskA)
            vt0 = dma_v_input(0)
            nc.gpsimd.dma_start(
                wv_sb[:], wvT[:].rearrange("(i p) f -> p i f", p=P))
            nc.gpsimd.dma_start(
                wo_sb[:], woT[:].rearrange("(i p) d -> p i d", p=P))

            # chunk 0 projection inline
            qk0, v0 = proj_tasks(0, tiles0, vt0)
            for task in qk0 + v0:
                task()

            deferred_v = None
            prefetched = {1: (dma_qk_inputs(1), dma_v_input(1))}
            for c in range(4):
                inner = deque()
                boundary = deque()
                if deferred_v is not None:
                    inner.extend(deferred_v)
                    deferred_v = None
                if c < 4 - 1:
                    nxt = c + 1
                    tiles, vt = prefetched.pop(nxt)
                    qk_t, v_t = proj_tasks(nxt, tiles, vt)
                    inner.extend(qk_t)
                    if nxt == 3:
                        deferred_v = v_t   # run inside attention(3) instead
                    else:
                        inner.extend(v_t)
                    if nxt + 1 <= 3:
                        def prefetch_task(cc=nxt + 1):
                            prefetched[cc] = (dma_qk_inputs(cc), dma_v_input(cc))
                        inner.insert(6, prefetch_task)
                if c >= 1:
                    for j in range(4):
                        boundary.append(lambda cc=c - 1, j=j: wo_task(cc, j))
                attention_chunk(c, inner, boundary, frontload=(c == 3))

            # final W_o for chunk 3 (psA slots are free now -> pipelined)
            for j in range(4):
                wo_task(3, j, pool=psA, ptag="b2")

    nc.compile()
    return nc


def _tables():
    inv = (1.0 / (ROPE_BASE ** (np.arange(0, Dh, 2, dtype=np.float32) / Dh))
           ).astype(np.float32)                      # [32]
    pos = np.arange(S, dtype=np.float32)
    ang = pos[:, None] * inv[None, :]                # [S, 32]
    cos = np.cos(ang).astype(np.float32)
    sin = np.sin(ang).astype(np.float32)
    d = np.arange(P) % Dh
    i = d // 2
    cosf = np.ascontiguousarray(cos[:, i].T)         # [128, S]
    sgn = np.where(d % 2 == 0, 1.0, -1.0).astype(np.float32)
    sinf = np.ascontiguousarray(sin[:, i].T * sgn[:, None]).astype(np.float32)

    p = np.arange(P)
    j = np.arange(P)
    maskA = np.where(p[:, None] <= j[None, :], 0.0, NEG).astype(np.float32)
    return cosf, sinf, maskA


def _core_inputs(q, k, v, W_q, W_k, W_v, W_o):
    """Build the 8 per-core input maps (bf16 transposed shards)."""
    cosf, sinf, maskA = _tables()
    in_maps = []
    for b in range(B):
        qTb = np.ascontiguousarray(q[b].T).astype(NPBF)
        kTb = np.ascontiguousarray(k[b].T).astype(NPBF)
        vTb = np.ascontiguousarray(v[b].T).astype(NPBF)
        for g in range(2):
            fs = slice(g * F, (g + 1) * F)
            in_maps.append({
                "qT": qTb, "kT": kTb, "vT": vTb,
                "wqT": np.ascontiguousarray(W_q[fs, :].T).astype(NPBF),
                "wkT": np.ascontiguousarray(W_k[fs, :].T).astype(NPBF),
                "wvT": np.ascontiguousarray(W_v[fs, :].T).astype(NPBF),
                "woT": np.ascontiguousarray(W_o[:, fs].T).astype(NPBF),
                "cosf": cosf, "sinf": sinf, "maskA": maskA,
            })
    return in_maps


def kernel(q, k, v, W_q, W_k, W_v, W_o):
    global _nc_cache, LAST_RESULT
    if _nc_cache is None:
        _nc_cache = _build_nc()
    nc = _nc_cache

    q = np.asarray(q, dtype=np.float32)
    k = np.asarray(k, dtype=np.float32)
    v = np.asarray(v, dtype=np.float32)
    W_q = np.asarray(W_q, dtype=np.float32)
    W_k = np.asarray(W_k, dtype=np.float32)
    W_v = np.asarray(W_v, dtype=np.float32)
    W_o = np.asarray(W_o, dtype=np.float32)

    in_maps = _core_inputs(q, k, v, W_q, W_k, W_v, W_o)

    res = bass_utils.run_bass_kernel_spmd(
        nc, in_maps, core_ids=list(range(N_CORES)), trace=KERNEL_TRACE)
    LAST_RESULT = res

    final = np.empty((B, S, D), dtype=np.float32)
    for b in range(B):
        final[b] = res.results[2 * b]["out"] + res.results[2 * b + 1]["out"]
    return final


# revision 11
# speedup vs baseline: 1.6970x; 1.0124x over previous
"""TRN2 Bass kernel for nn_Attention_23493471109551 — v2.

Full attention layer: QKV projections + interleaved RoPE + causal softmax
attention + output projection, for B=4, S=2048, D=1024, H=16, Dh=64.

Sharding: 8 cores = 4 batches x 2 head-groups (8 heads each).  Each core
computes its batch/head-group's attention and a partial output projection
(W_o row-block); host sums the two partials per batch.

v2 vs v1:
  * bf16 everywhere on the matmul paths (1 cyc/row at ANY moving size;
    kills the fp32r <256-col 4x penalty), fp32 PSUM accumulation.
  * all weights persist in SBUF (loaded once; v1 re-read 33MB from HBM).
  * chunk-pipelined schedule: projection matmuls of chunk c+1 and the
    W_o matmuls of chunk c-1 are interleaved into attention(c)'s
    score->exp->PV loop as PE filler so the PE never idles long enough
    for the HAM clock-gate to re-throttle, and the ACT-bound exp stream
    overlaps PE work instead of serializing with it.
  * software-pipelined attention inner loop (score(t+1) issued before
    PV(t)) so PV's wait on exp(t) is hidden.
  * reciprocal_approx_fast for softmax denominators (v1 used full-precision
    DVE reciprocal: 3.3us per call, 106us total).
"""
import math
from collections import deque

import numpy as np
import ml_dtypes

import concourse.bass as bass
import concourse.tile as tile
import concourse.mybir as mybir
from concourse import bacc, bass_utils

# problem constants
B, S, D = 4, 2048, 1024
H, Dh = 16, 64
EQ, EV = 2048, 1024          # q/k and v input feature dims
F = 512                      # features per core (8 heads x 64)
P = 128
N_CORES = 8
SCALE = 1.0 / math.sqrt(D)   # 1/32
ROPE_BASE = 10000.0
NEG = -1.0e30
SWAP_MASK = [i ^ 1 for i in range(32)]

F32 = mybir.dt.float32
BF16 = mybir.dt.bfloat16
NPBF = ml_dtypes.bfloat16

# test hooks (harness ignores these)
KERNEL_TRACE = False
LAST_RESULT = None

_nc_cache = None


def _build_nc():
    nc = bacc.Bacc("TRN2", target_bir_lowering=False, debug=False)
    qT = nc.dram_tensor("qT", [EQ, S], BF16, kind="ExternalInput").ap()
    kT = nc.dram_tensor("kT", [EQ, S], BF16, kind="ExternalInput").ap()
    vT = nc.dram_tensor("vT", [EV, S], BF16, kind="ExternalInput").ap()
    wqT = nc.dram_tensor("wqT", [EQ, F], BF16, kind="ExternalInput").ap()
    wkT = nc.dram_tensor("wkT", [EQ, F], BF16, kind="ExternalInput").ap()
    wvT = nc.dram_tensor("wvT", [EV, F], BF16, kind="ExternalInput").ap()
    woT = nc.dram_tensor("woT", [F, D], BF16, kind="ExternalInput").ap()
    cosf = nc.dram_tensor("cosf", [P, S], F32, kind="ExternalInput").ap()
    sinf = nc.dram_tensor("sinf", [P, S], F32, kind="ExternalInput").ap()
    maskA = nc.dram_tensor("maskA", [P, P], F32, kind="ExternalInput").ap()
    out = nc.dram_tensor("out", [S, D], F32, kind="ExternalOutput").ap()

    EXP = mybir.ActivationFunctionType.Exp

    with tile.TileContext(nc) as tc:
        with (
            tc.tile_pool(name="consts", bufs=1) as consts,
            tc.tile_pool(name="persist", bufs=1) as persist,
            tc.tile_pool(name="qin", bufs=3) as qin_pool,
            tc.tile_pool(name="vin", bufs=2) as vin_pool,
            tc.tile_pool(name="rope", bufs=2) as rope_pool,
            tc.tile_pool(name="expp", bufs=3) as et_pool,
            tc.tile_pool(name="norm", bufs=2) as norm_pool,
            tc.tile_pool(name="attnc", bufs=3) as attnc_pool,
            tc.tile_pool(name="outsb", bufs=2) as out_pool,
            tc.tile_pool(name="psA", bufs=2, space="PSUM") as psA,
            tc.tile_pool(name="psB", bufs=1, space="PSUM") as psB,
        ):
            # ---- persistent SBUF
            wq_sb = consts.tile([P, 16, F], BF16, tag="wq")
            wk_sb = consts.tile([P, 16, F], BF16, tag="wk")
            wv_sb = consts.tile([P, 8, F], BF16, tag="wv")
            wo_sb = consts.tile([P, 4, D], BF16, tag="wo")
            cos_t = consts.tile([P, S], F32, tag="cos")
            sin_t = consts.tile([P, S], F32, tag="sin")
            mask_t = consts.tile([P, P], F32, tag="mask")

            qpT = persist.tile([P, 4, S], BF16, tag="qpT")
            kpT = persist.tile([P, 4, S], BF16, tag="kpT")
            vpa = persist.tile([P, 16, 8, 65], BF16, tag="vpa")

            nc.vector.memset(vpa[:], 1.0)  # ones col; 0:64 overwritten

            # PE warm-up: ~4us of dummy matmuls on a zeroed tile so the HAM
            # clock-gate reaches K=8/8 while the first input DMAs stream in.
            warm = consts.tile([P, 512], BF16, tag="warm")
            nc.vector.memset(warm[:], 0.0)
            wps = psA.tile([P, 2, 512], F32, tag="b2", name="warmps")
            for wi in range(20):
                nc.tensor.matmul(wps[:, wi % 2, :], warm[:, 0:P], warm[:],
                                 start=True, stop=True)

            # ---- input DMA helpers -------------------------------------
            def dma_qk_inputs(c):
                """Start DMAs for chunk c's q/k e-tiles (2 halves each)."""
                ssl = slice(c * 512, (c + 1) * 512)
                tiles = {}
                for key, src, eng in (("q", qT, nc.sync), ("k", kT, nc.scalar)):
                    hs = []
                    for h in range(2):
                        t = qin_pool.tile([P, 8, 512], BF16, tag="qin",
                                          name=f"{key}in{c}h{h}")
                        eng.dma_start(
                            t[:],
                            src[h * 1024:(h + 1) * 1024, ssl]
                            .rearrange("(i p) s -> p i s", p=P))
                        hs.append(t)
                    tiles[key] = hs
                return tiles

            def dma_v_input(c):
                ssl = slice(c * 512, (c + 1) * 512)
                t = vin_pool.tile([P, 8, 512], BF16, tag="vin", name=f"vin{c}")
                nc.gpsimd.dma_start(
                    t[:], vT[:, ssl].rearrange("(i p) s -> p i s", p=P))
                return t

            # ---- projection task generators ----------------------------
            def qk_mm_task(w_sb, tiles, cp, half, pstate):
                if half == 0:
                    pstate["ps"] = psA.tile([P, 2, 512], F32, tag="pq", bufs=1,
                                            name="psq")
                ps = pstate["ps"]
                for e in range(half * 8, half * 8 + 8):
                    stile = tiles[e // 8]
                    for i in range(2):
                        ci = 2 * cp + i
                        nc.tensor.matmul(
                            ps[:, i, :],
                            w_sb[:, e, ci * P:(ci + 1) * P],
                            stile[:, e % 8, :],
                            start=(e == 0), stop=(e == 15))

            def rope_task(dstT, cp, pstate, ssl):
                ps = pstate["ps"]
                a = rope_pool.tile([P, 2, 512], BF16, tag="ra")
                nc.vector.tensor_mul(
                    a[:], ps[:], cos_t[:, None, ssl].to_broadcast((P, 2, 512)))
                cc_ = rope_pool.tile([P, 2, 512], BF16, tag="rb")
                nc.vector.tensor_mul(
                    cc_[:], ps[:], sin_t[:, None, ssl].to_broadcast((P, 2, 512)))
                c2 = rope_pool.tile([P, 2, 512], BF16, tag="rc2")
                nc.vector.stream_shuffle(c2[:], cc_[:], SWAP_MASK)
                nc.vector.tensor_add(dstT[:, 2 * cp:2 * cp + 2, ssl], a[:], c2[:])

            def v_mm_task(vt, jp, half, pstate):
                if half == 0:
                    pstate["ps"] = psA.tile([P, 2, 512], F32, tag="pq", bufs=1,
                                            name="psv")
                ps = pstate["ps"]
                for e in range(half * 4, half * 4 + 4):
                    for i in range(2):
                        j = 2 * jp + i
                        nc.tensor.matmul(
                            ps[:, i, :],
                            vt[:, e, j * P:(j + 1) * P],
                            wv_sb[:, e, :],
                            start=(e == 0), stop=(e == 7))

            def v_copy_task(c, jp, pstate):
                ps = pstate["ps"]
                for i in range(2):
                    st = c * 4 + 2 * jp + i
                    nc.vector.tensor_copy(
                        vpa[:, st, :, 0:64],
                        ps[:, i, :].rearrange("p (h d) -> p h d", h=8))

            def proj_tasks(c, tiles, vt):
                """(qk_tasks, v_tasks) emission closures projecting chunk c."""
                ssl = slice(c * 512, (c + 1) * 512)
                tasks = []
                for (w_sb, dstT, key) in ((wq_sb, qpT, "q"), (wk_sb, kpT, "k")):
                    for cp in range(2):
                        st = {}
                        tasks.append(
                            lambda w=w_sb, t=tiles[key], cp=cp, s=st:
                                qk_mm_task(w, t, cp, 0, s))
                        tasks.append(
                            lambda w=w_sb, t=tiles[key], cp=cp, s=st:
                                qk_mm_task(w, t, cp, 1, s))
                        tasks.append(
                            lambda d=dstT, cp=cp, s=st: rope_task(d, cp, s, ssl))
                vtasks = []
                for jp in range(2):
                    st = {}
                    vtasks.append(lambda v=vt, jp=jp, s=st: v_mm_task(v, jp, 0, s))
                    vtasks.append(lambda v=vt, jp=jp, s=st: v_mm_task(v, jp, 1, s))
                    vtasks.append(lambda jp=jp, s=st: v_copy_task(c, jp, s))
                return tasks, vtasks

            # ---- W_o task ----------------------------------------------
            attn_tiles = {}

            def wo_task(cc, j, pool=None, ptag="po"):
                at = attn_tiles[cc]
                pw = (pool or psB).tile([P, 2, 512], F32, tag=ptag,
                                        name=f"pw{cc}_{j}")
                for ci in range(4):
                    for oc in range(2):
                        nc.tensor.matmul(
                            pw[:, oc, :],
                            at[:, ci, j * P:(j + 1) * P],
                            wo_sb[:, ci, oc * 512:(oc + 1) * 512],
                            start=(ci == 0), stop=(ci == 3))
                ot = out_pool.tile([P, 2, 512], F32, tag="osb")
                if j % 2 == 0:
                    nc.vector.tensor_copy(ot[:], pw[:])
                else:
                    nc.scalar.copy(ot[:], pw[:])
                row = (4 * cc + j) * P
                (nc.gpsimd if j % 2 == 0 else nc.sync).dma_start(
                    out[row:row + P, :], ot[:].rearrange("p a b -> p (a b)"))

            # ---- attention chunk ---------------------------------------
            def attention_chunk(c, inner, boundary, frontload=False):
                """inner: deque of filler closures run between t-iterations.
                boundary: deque run at hp-group boundaries (need psB free).
                frontload: drain inner at 1/iter from the start (needed when
                fillers write tiles read later in THIS chunk)."""
                nt = 4 * (c + 1)
                attn_c = attnc_pool.tile([P, 4, 512], BF16, tag="attn_c",
                                         name=f"attn{c}")
                attn_tiles[c] = attn_c
                n_iters = 4 * nt
                n_inner = len(inner)
                bper = (len(boundary) + 3) // 4
                done = 0
                it = 0
                for hp in range(4):
                    po = psB.tile([P, 2, 512], F32, tag="po", name=f"po{c}_{hp}")
                    prev = None
                    for t in range(nt):
                        tsl = slice(t * P, (t + 1) * P)
                        rr = 128 * (t - 4 * c) if t >= 4 * c else 0
                        qsl = slice(c * 512 + rr, (c + 1) * 512)
                        ps_s = psA.tile([P, 2, 512], F32, tag="b2")
                        nc.tensor.matmul(ps_s[:, 0, rr:512],
                                         kpT[0:64, hp, tsl], qpT[0:64, hp, qsl],
                                         start=True, stop=True)
                        nc.tensor.matmul(ps_s[:, 1, rr:512],
                                         kpT[64:128, hp, tsl],
                                         qpT[64:128, hp, qsl],
                                         start=True, stop=True)
                        if t >= 4 * c:
                            nc.vector.tensor_add(
                                ps_s[:, :, rr:rr + 128], ps_s[:, :, rr:rr + 128],
                                mask_t[:, None, :].to_broadcast((P, 2, P)))
                        et = et_pool.tile([P, 2, 512], BF16, tag="exp")
                        nc.scalar.activation(et[:, :, rr:512], ps_s[:, :, rr:512],
                                             EXP, scale=SCALE)
                        if prev is not None:
                            emit_pv(c, hp, po, prev, nt)
                        # proportional filler pacing
                        it += 1
                        want = it if frontload else (it * n_inner) // n_iters
                        while done < want and inner:
                            inner.popleft()()
                            done += 1
                        prev = (t, et, rr)
                    emit_pv(c, hp, po, prev, nt)
                    # normalize hp: denominators are po[64]
                    den = norm_pool.tile([1, 2, 512], F32, tag="den", bufs=1)
                    nc.vector.tensor_copy(den[:], po[64:65, :, :])
                    rc = norm_pool.tile([1, 2, 512], F32, tag="rc", bufs=1)
                    nc.vector.reciprocal_approx_fast(rc[:], den[:])
                    bc = norm_pool.tile([64, 2, 512], F32, tag="bc", bufs=1)
                    nc.gpsimd.partition_broadcast(bc[:], rc[:])
                    nc.vector.tensor_mul(attn_c[0:64, hp, :],
                                         po[0:64, 0, :], bc[:, 0, :])
                    nc.vector.tensor_mul(attn_c[64:128, hp, :],
                                         po[0:64, 1, :], bc[:, 1, :])
                    for _ in range(bper):
                        if boundary:
                            boundary.popleft()()
                while inner:
                    inner.popleft()()
                while boundary:
                    boundary.popleft()()

            def emit_pv(c, hp, po, prev, nt):
                t, et, rr = prev
                nc.tensor.matmul(po[0:65, 0, rr:512], vpa[:, t, 2 * hp, :],
                                 et[:, 0, rr:512],
                                 start=(t == 0), stop=(t == nt - 1))
                nc.tensor.matmul(po[0:65, 1, rr:512], vpa[:, t, 2 * hp + 1, :],
                                 et[:, 1, rr:512],
                                 start=(t == 0), stop=(t == nt - 1))

            # ---- main schedule -----------------------------------------
            # chunk 0 inputs + weights (interleaved so first MMs start early)
            tiles0 = {}
            ssl0 = slice(0, 512)
            tiles0["q"] = []
            for h in range(2):
                t = qin_pool.tile([P, 8, 512], BF16, tag="qin", name=f"qin0h{h}")
                nc.sync.dma_start(
                    t[:], qT[h * 1024:(h + 1) * 1024, ssl0]
                    .rearrange("(i p) s -> p i s", p=P))
                nc.sync.dma_start(
                    wq_sb[:, 8 * h:8 * h + 8, :],
                    wqT[h * 1024:(h + 1) * 1024, :]
                    .rearrange("(i p) f -> p i f", p=P))
                tiles0["q"].append(t)
            tiles0["k"] = []
            for h in range(2):
                t = qin_pool.tile([P, 8, 512], BF16, tag="qin", name=f"kin0h{h}")
                nc.scalar.dma_start(
                    t[:], kT[h * 1024:(h + 1) * 1024, ssl0]
                    .rearrange("(i p) s -> p i s", p=P))
                nc.scalar.dma_start(
                    wk_sb[:, 8 * h:8 * h + 8, :],
                    wkT[h * 1024:(h + 1) * 1024, :]
                    .rearrange("(i p) f -> p i f", p=P))
                tiles0["k"].append(t)
            # consts + v on the gpsimd queue
            nc.gpsimd.dma_start(cos_t[:], cosf)
            nc.gpsimd.dma_start(sin_t[:], sinf)
            nc.gpsimd.dma_start(mask_t[:], maskA)
            vt0 = dma_v_input(0)
            nc.gpsimd.dma_start(
                wv_sb[:], wvT[:].rearrange("(i p) f -> p i f", p=P))
            nc.gpsimd.dma_start(
                wo_sb[:], woT[:].rearrange("(i p) d -> p i d", p=P))

            # chunk 0 projection inline; spin the PE through the k/v DMA
            # waits so the HAM clock-gate stays open
            qk0, v0 = proj_tasks(0, tiles0, vt0)
            for ti, task in enumerate(qk0 + v0):
                task()
                if ti == 5:
                    for wi in range(16):
                        nc.tensor.matmul(wps[:, wi % 2, 0:256], warm[:, 0:P],
                                         warm[:, 0:256], start=True, stop=True)
                elif ti == 11:
                    for wi in range(8):
                        nc.tensor.matmul(wps[:, wi % 2, 0:256], warm[:, 0:P],
                                         warm[:, 0:256], start=True, stop=True)

            deferred_v = None
            prefetched = {1: (dma_qk_inputs(1), dma_v_input(1))}
            for c in range(4):
                inner = deque()
                boundary = deque()
                if deferred_v is not None:
                    inner.extend(deferred_v)
                    deferred_v = None
                if c < 4 - 1:
                    nxt = c + 1
                    tiles, vt = prefetched.pop(nxt)
                    qk_t, v_t = proj_tasks(nxt, tiles, vt)
                    inner.extend(qk_t)
                    if nxt == 3:
                        deferred_v = v_t   # run inside attention(3) instead
                    else:
                        inner.extend(v_t)
                    if nxt + 1 <= 3:
                        def prefetch_task(cc=nxt + 1):
                            prefetched[cc] = (dma_qk_inputs(cc), dma_v_input(cc))
                        inner.insert(6, prefetch_task)
                for cc in {1: [0], 3: [1, 2]}.get(c, []):
                    for j in range(4):
                        boundary.append(lambda cc=cc, j=j: wo_task(cc, j))
                attention_chunk(c, inner, boundary, frontload=(c == 3))

            # final W_o for chunk 3 (psA slots are free now -> pipelined)
            for j in range(4):
                wo_task(3, j, pool=psA, ptag="b2")

    nc.compile()
    return nc


def _tables():
    inv = (1.0 / (ROPE_BASE ** (np.arange(0, Dh, 2, dtype=np.float32) / Dh))
           ).astype(np.float32)                      # [32]
    pos = np.arange(S, dtype=np.float32)
    ang = pos[:, None] * inv[None, :]                # [S, 32]
    cos = np.cos(ang).astype(np.float32)
    sin = np.sin(ang).astype(np.float32)
    d = np.arange(P) % Dh
    i = d // 2
    cosf = np.ascontiguousarray(cos[:, i].T)         # [128, S]
    sgn = np.where(d % 2 == 0, 1.0, -1.0).astype(np.float32)
    sinf = np.ascontiguousarray(sin[:, i].T * sgn[:, None]).astype(np.float32)

    p = np.arange(P)
    j = np.arange(P)
    maskA = np.where(p[:, None] <= j[None, :], 0.0, NEG).astype(np.float32)
    return cosf, sinf, maskA


def _core_inputs(q, k, v, W_q, W_k, W_v, W_o):
    """Build the 8 per-core input maps (bf16 transposed shards)."""
    cosf, sinf, maskA = _tables()
    in_maps = []
    for b in range(B):
        qTb = np.ascontiguousarray(q[b].T).astype(NPBF)
        kTb = np.ascontiguousarray(k[b].T).astype(NPBF)
        vTb = np.ascontiguousarray(v[b].T).astype(NPBF)
        for g in range(2):
            fs = slice(g * F, (g + 1) * F)
            in_maps.append({
                "qT": qTb, "kT": kTb, "vT": vTb,
                "wqT": np.ascontiguousarray(W_q[fs, :].T).astype(NPBF),
                "wkT": np.ascontiguousarray(W_k[fs, :].T).astype(NPBF),
                "wvT": np.ascontiguousarray(W_v[fs, :].T).astype(NPBF),
                "woT": np.ascontiguousarray(W_o[:, fs].T).astype(NPBF),
                "cosf": cosf, "sinf": sinf, "maskA": maskA,
            })
    return in_maps


def kernel(q, k, v, W_q, W_k, W_v, W_o):
    global _nc_cache, LAST_RESULT
    if _nc_cache is None:
        _nc_cache = _build_nc()
    nc = _nc_cache

    q = np.asarray(q, dtype=np.float32)
    k = np.asarray(k, dtype=np.float32)
    v = np.asarray(v, dtype=np.float32)
    W_q = np.asarray(W_q, dtype=np.float32)
    W_k = np.asarray(W_k, dtype=np.float32)
    W_v = np.asarray(W_v, dtype=np.float32)
    W_o = np.asarray(W_o, dtype=np.float32)

    in_maps = _core_inputs(q, k, v, W_q, W_k, W_v, W_o)

    res = bass_utils.run_bass_kernel_spmd(
        nc, in_maps, core_ids=list(range(N_CORES)), trace=KERNEL_TRACE)
    LAST_RESULT = res

    final = np.empty((B, S, D), dtype=np.float32)
    for b in range(B):
        final[b] = res.results[2 * b]["out"] + res.results[2 * b + 1]["out"]
    return final


# revision 13
# speedup vs baseline: 1.7320x; 1.0206x over previous
"""TRN2 Bass kernel for nn_Attention_23493471109551 — v2.

Full attention layer: QKV projections + interleaved RoPE + causal softmax
attention + output projection, for B=4, S=2048, D=1024, H=16, Dh=64.

Sharding: 8 cores = 4 batches x 2 head-groups (8 heads each).  Each core
computes its batch/head-group's attention and a partial output projection
(W_o row-block); host sums the two partials per batch.

v2 vs v1:
  * bf16 everywhere on the matmul paths (1 cyc/row at ANY moving size;
    kills the fp32r <256-col 4x penalty), fp32 PSUM accumulation.
  * all weights persist in SBUF (loaded once; v1 re-read 33MB from HBM).
  * chunk-pipelined schedule: projection matmuls of chunk c+1 and the
    W_o matmuls of chunk c-1 are interleaved into attention(c)'s
    score->exp->PV loop as PE filler so the PE never idles long enough
    for the HAM clock-gate to re-throttle, and the ACT-bound exp stream
    overlaps PE work instead of serializing with it.
  * software-pipelined attention inner loop (score(t+1) issued before
    PV(t)) so PV's wait on exp(t) is hidden.
  * reciprocal_approx_fast for softmax denominators (v1 used full-precision
    DVE reciprocal: 3.3us per call, 106us total).
"""
import math
from collections import deque

import numpy as np
import ml_dtypes

import concourse.bass as bass
import concourse.tile as tile
import concourse.mybir as mybir
from concourse import bacc, bass_utils

# problem constants
B, S, D = 4, 2048, 1024
H, Dh = 16, 64
EQ, EV = 2048, 1024          # q/k and v input feature dims
F = 512                      # features per core (8 heads x 64)
P = 128
N_CORES = 8
SCALE = 1.0 / math.sqrt(D)   # 1/32
ROPE_BASE = 10000.0
NEG = -1.0e30
SWAP_MASK = [i ^ 1 for i in range(32)]

F32 = mybir.dt.float32
BF16 = mybir.dt.bfloat16
NPBF = ml_dtypes.bfloat16

# test hooks (harness ignores these)
KERNEL_TRACE = False
LAST_RESULT = None

_nc_cache = None


def _build_nc():
    nc = bacc.Bacc("TRN2", target_bir_lowering=False, debug=False)
    qT = nc.dram_tensor("qT", [EQ, S], BF16, kind="ExternalInput").ap()
    kT = nc.dram_tensor("kT", [EQ, S], BF16, kind="ExternalInput").ap()
    vT = nc.dram_tensor("vT", [EV, S], BF16, kind="ExternalInput").ap()
    wqT = nc.dram_tensor("wqT", [EQ, F], BF16, kind="ExternalInput").ap()
    wkT = nc.dram_tensor("wkT", [EQ, F], BF16, kind="ExternalInput").ap()
    wvT = nc.dram_tensor("wvT", [EV, F], BF16, kind="ExternalInput").ap()
    woT = nc.dram_tensor("woT", [F, D], BF16, kind="ExternalInput").ap()
    cosf = nc.dram_tensor("cosf", [P, S], F32, kind="ExternalInput").ap()
    sinf = nc.dram_tensor("sinf", [P, S], F32, kind="ExternalInput").ap()
    maskA = nc.dram_tensor("maskA", [P, P], F32, kind="ExternalInput").ap()
    out = nc.dram_tensor("out", [S, D], F32, kind="ExternalOutput").ap()

    EXP = mybir.ActivationFunctionType.Exp

    with tile.TileContext(nc) as tc:
        with (
            tc.tile_pool(name="consts", bufs=1) as consts,
            tc.tile_pool(name="persist", bufs=1) as persist,
            tc.tile_pool(name="qin", bufs=3) as qin_pool,
            tc.tile_pool(name="vin", bufs=2) as vin_pool,
            tc.tile_pool(name="rope", bufs=2) as rope_pool,
            tc.tile_pool(name="expp", bufs=3) as et_pool,
            tc.tile_pool(name="norm", bufs=2) as norm_pool,
            tc.tile_pool(name="attnc", bufs=3) as attnc_pool,
            tc.tile_pool(name="outsb", bufs=2) as out_pool,
            tc.tile_pool(name="psA", bufs=2, space="PSUM") as psA,
            tc.tile_pool(name="psB", bufs=1, space="PSUM") as psB,
        ):
            # ---- persistent SBUF
            wq_sb = consts.tile([P, 16, F], BF16, tag="wq")
            wk_sb = consts.tile([P, 16, F], BF16, tag="wk")
            wv_sb = consts.tile([P, 8, F], BF16, tag="wv")
            wo_sb = consts.tile([P, 4, D], BF16, tag="wo")
            cos_t = consts.tile([P, S], F32, tag="cos")
            sin_t = consts.tile([P, S], F32, tag="sin")
            mask_t = consts.tile([P, P], F32, tag="mask")

            qpT = persist.tile([P, 4, S], BF16, tag="qpT")
            kpT = persist.tile([P, 4, S], BF16, tag="kpT")
            vpa = persist.tile([P, 16, 8, 65], BF16, tag="vpa")

            nc.vector.memset(vpa[:], 1.0)  # ones col; 0:64 overwritten
            mask01 = consts.tile([P, P], BF16, tag="mask01")

            # PE warm-up: ~4us of dummy matmuls on a zeroed tile so the HAM
            # clock-gate reaches K=8/8 while the first input DMAs stream in.
            warm = consts.tile([P, 512], BF16, tag="warm")
            nc.vector.memset(warm[:], 0.0)
            wps = psA.tile([P, 2, 512], F32, tag="b2", name="warmps")
            for wi in range(20):
                nc.tensor.matmul(wps[:, wi % 2, :], warm[:, 0:P], warm[:],
                                 start=True, stop=True)

            # ---- input DMA helpers -------------------------------------
            def dma_qk_inputs(c):
                """Start DMAs for chunk c's q/k e-tiles (2 halves each)."""
                ssl = slice(c * 512, (c + 1) * 512)
                tiles = {}
                for key, src, eng in (("q", qT, nc.sync), ("k", kT, nc.scalar)):
                    hs = []
                    for h in range(2):
                        t = qin_pool.tile([P, 8, 512], BF16, tag="qin",
                                          name=f"{key}in{c}h{h}")
                        eng.dma_start(
                            t[:],
                            src[h * 1024:(h + 1) * 1024, ssl]
                            .rearrange("(i p) s -> p i s", p=P))
                        hs.append(t)
                    tiles[key] = hs
                return tiles

            def dma_v_input(c):
                ssl = slice(c * 512, (c + 1) * 512)
                t = vin_pool.tile([P, 8, 512], BF16, tag="vin", name=f"vin{c}")
                nc.gpsimd.dma_start(
                    t[:], vT[:, ssl].rearrange("(i p) s -> p i s", p=P))
                return t

            # ---- projection task generators ----------------------------
            def qk_mm_task(w_sb, tiles, cp, half, pstate):
                if half == 0:
                    pstate["ps"] = psA.tile([P, 2, 512], F32, tag="pq", bufs=1,
                                            name="psq")
                ps = pstate["ps"]
                for e in range(half * 8, half * 8 + 8):
                    stile = tiles[e // 8]
                    for i in range(2):
                        ci = 2 * cp + i
                        nc.tensor.matmul(
                            ps[:, i, :],
                            w_sb[:, e, ci * P:(ci + 1) * P],
                            stile[:, e % 8, :],
                            start=(e == 0), stop=(e == 15))

            def rope_task(dstT, cp, pstate, ssl):
                ps = pstate["ps"]
                a = rope_pool.tile([P, 2, 512], BF16, tag="ra")
                nc.vector.tensor_mul(
                    a[:], ps[:], cos_t[:, None, ssl].to_broadcast((P, 2, 512)))
                cc_ = rope_pool.tile([P, 2, 512], BF16, tag="rb")
                nc.vector.tensor_mul(
                    cc_[:], ps[:], sin_t[:, None, ssl].to_broadcast((P, 2, 512)))
                c2 = rope_pool.tile([P, 2, 512], BF16, tag="rc2")
                nc.vector.stream_shuffle(c2[:], cc_[:], SWAP_MASK)
                nc.vector.tensor_add(dstT[:, 2 * cp:2 * cp + 2, ssl], a[:], c2[:])

            def v_mm_task(vt, jp, half, pstate):
                if half == 0:
                    pstate["ps"] = psA.tile([P, 2, 512], F32, tag="pq", bufs=1,
                                            name="psv")
                ps = pstate["ps"]
                for e in range(half * 4, half * 4 + 4):
                    for i in range(2):
                        j = 2 * jp + i
                        nc.tensor.matmul(
                            ps[:, i, :],
                            vt[:, e, j * P:(j + 1) * P],
                            wv_sb[:, e, :],
                            start=(e == 0), stop=(e == 7))

            def v_copy_task(c, jp, pstate):
                ps = pstate["ps"]
                for i in range(2):
                    st = c * 4 + 2 * jp + i
                    nc.vector.tensor_copy(
                        vpa[:, st, :, 0:64],
                        ps[:, i, :].rearrange("p (h d) -> p h d", h=8))

            def proj_tasks(c, tiles, vt):
                """(qk_tasks, v_tasks) emission closures projecting chunk c."""
                ssl = slice(c * 512, (c + 1) * 512)
                tasks = []
                for (w_sb, dstT, key) in ((wq_sb, qpT, "q"), (wk_sb, kpT, "k")):
                    for cp in range(2):
                        st = {}
                        tasks.append(
                            lambda w=w_sb, t=tiles[key], cp=cp, s=st:
                                qk_mm_task(w, t, cp, 0, s))
                        tasks.append(
                            lambda w=w_sb, t=tiles[key], cp=cp, s=st:
                                qk_mm_task(w, t, cp, 1, s))
                        tasks.append(
                            lambda d=dstT, cp=cp, s=st: rope_task(d, cp, s, ssl))
                vtasks = []
                for jp in range(2):
                    st = {}
                    vtasks.append(lambda v=vt, jp=jp, s=st: v_mm_task(v, jp, 0, s))
                    vtasks.append(lambda v=vt, jp=jp, s=st: v_mm_task(v, jp, 1, s))
                    vtasks.append(lambda jp=jp, s=st: v_copy_task(c, jp, s))
                return tasks, vtasks

            # ---- W_o task ----------------------------------------------
            attn_tiles = {}

            def wo_task(cc, j, pool=None, ptag="po"):
                at = attn_tiles[cc]
                pw = (pool or psB).tile([P, 2, 512], F32, tag=ptag,
                                        name=f"pw{cc}_{j}")
                for ci in range(4):
                    for oc in range(2):
                        nc.tensor.matmul(
                            pw[:, oc, :],
                            at[:, ci, j * P:(j + 1) * P],
                            wo_sb[:, ci, oc * 512:(oc + 1) * 512],
                            start=(ci == 0), stop=(ci == 3))
                ot = out_pool.tile([P, 2, 512], F32, tag="osb")
                if j % 2 == 0:
                    nc.vector.tensor_copy(ot[:], pw[:])
                else:
                    nc.scalar.copy(ot[:], pw[:])
                row = (4 * cc + j) * P
                (nc.gpsimd if j % 2 == 0 else nc.sync).dma_start(
                    out[row:row + P, :], ot[:].rearrange("p a b -> p (a b)"))

            # ---- attention chunk ---------------------------------------
            def attention_chunk(c, inner, boundary, frontload=False):
                """inner: deque of filler closures run between t-iterations.
                boundary: deque run at hp-group boundaries (need psB free).
                frontload: drain inner at 1/iter from the start (needed when
                fillers write tiles read later in THIS chunk)."""
                nt = 4 * (c + 1)
                attn_c = attnc_pool.tile([P, 4, 512], BF16, tag="attn_c",
                                         name=f"attn{c}")
                attn_tiles[c] = attn_c
                n_iters = 4 * nt
                n_inner = len(inner)
                bper = (len(boundary) + 3) // 4
                done = 0
                it = 0
                for hp in range(4):
                    po = psB.tile([P, 2, 512], F32, tag="po", name=f"po{c}_{hp}")
                    prev = None
                    for t in range(nt):
                        tsl = slice(t * P, (t + 1) * P)
                        rr = 128 * (t - 4 * c) if t >= 4 * c else 0
                        qsl = slice(c * 512 + rr, (c + 1) * 512)
                        # in attention(3) the proj-psum ring is idle: borrow
                        # it as a 3rd score buffer so ACT never starves
                        use_pq = (c == 3 and t % 3 == 2)
                        ps_s = psA.tile([P, 2, 512], F32,
                                        tag=("pq" if use_pq else "b2"),
                                        bufs=(1 if use_pq else None),
                                        name="ps_s")
                        nc.tensor.matmul(ps_s[:, 0, rr:512],
                                         kpT[0:64, hp, tsl], qpT[0:64, hp, qsl],
                                         start=True, stop=True)
                        nc.tensor.matmul(ps_s[:, 1, rr:512],
                                         kpT[64:128, hp, tsl],
                                         qpT[64:128, hp, qsl],
                                         start=True, stop=True)
                        et = et_pool.tile([P, 2, 512], BF16, tag="exp")
                        nc.scalar.activation(et[:, :, rr:512], ps_s[:, :, rr:512],
                                             EXP, scale=SCALE)
                        if t >= 4 * c:
                            # zero the non-causal upper triangle post-exp (off
                            # the ACT critical path; PV waits on this instead)
                            nc.vector.tensor_mul(
                                et[:, :, rr:rr + 128], et[:, :, rr:rr + 128],
                                mask01[:, None, :].to_broadcast((P, 2, P)))
                        if prev is not None:
                            emit_pv(c, hp, po, prev, nt)
                        # proportional filler pacing
                        it += 1
                        want = it if frontload else (it * n_inner) // n_iters
                        while done < want and inner:
                            inner.popleft()()
                            done += 1
                        prev = (t, et, rr)
                    emit_pv(c, hp, po, prev, nt)
                    # normalize hp: denominators are po[64]
                    den = norm_pool.tile([1, 2, 512], F32, tag="den", bufs=1)
                    nc.vector.tensor_copy(den[:], po[64:65, :, :])
                    rc = norm_pool.tile([1, 2, 512], F32, tag="rc", bufs=1)
                    nc.vector.reciprocal_approx_fast(rc[:], den[:])
                    bc = norm_pool.tile([64, 2, 512], F32, tag="bc", bufs=1)
                    nc.gpsimd.partition_broadcast(bc[:], rc[:])
                    nc.vector.tensor_mul(attn_c[0:64, hp, :],
                                         po[0:64, 0, :], bc[:, 0, :])
                    nc.vector.tensor_mul(attn_c[64:128, hp, :],
                                         po[0:64, 1, :], bc[:, 1, :])
                    for _ in range(bper):
                        if boundary:
                            boundary.popleft()()
                while inner:
                    inner.popleft()()
                while boundary:
                    boundary.popleft()()

            def emit_pv(c, hp, po, prev, nt):
                t, et, rr = prev
                nc.tensor.matmul(po[0:65, 0, rr:512], vpa[:, t, 2 * hp, :],
                                 et[:, 0, rr:512],
                                 start=(t == 0), stop=(t == nt - 1))
                nc.tensor.matmul(po[0:65, 1, rr:512], vpa[:, t, 2 * hp + 1, :],
                                 et[:, 1, rr:512],
                                 start=(t == 0), stop=(t == nt - 1))

            # ---- main schedule -----------------------------------------
            # chunk 0 inputs + weights (interleaved so first MMs start early)
            tiles0 = {}
            ssl0 = slice(0, 512)
            tiles0["q"] = []
            for h in range(2):
                t = qin_pool.tile([P, 8, 512], BF16, tag="qin", name=f"qin0h{h}")
                nc.sync.dma_start(
                    t[:], qT[h * 1024:(h + 1) * 1024, ssl0]
                    .rearrange("(i p) s -> p i s", p=P))
                nc.sync.dma_start(
                    wq_sb[:, 8 * h:8 * h + 8, :],
                    wqT[h * 1024:(h + 1) * 1024, :]
                    .rearrange("(i p) f -> p i f", p=P))
                tiles0["q"].append(t)
            tiles0["k"] = []
            for h in range(2):
                t = qin_pool.tile([P, 8, 512], BF16, tag="qin", name=f"kin0h{h}")
                nc.scalar.dma_start(
                    t[:], kT[h * 1024:(h + 1) * 1024, ssl0]
                    .rearrange("(i p) s -> p i s", p=P))
                nc.scalar.dma_start(
                    wk_sb[:, 8 * h:8 * h + 8, :],
                    wkT[h * 1024:(h + 1) * 1024, :]
                    .rearrange("(i p) f -> p i f", p=P))
                tiles0["k"].append(t)
            # consts + v on the gpsimd queue
            nc.gpsimd.dma_start(cos_t[:], cosf)
            nc.gpsimd.dma_start(sin_t[:], sinf)
            nc.gpsimd.dma_start(mask_t[:], maskA)
            # 0/1 causal mask (bf16) for post-exp zeroing of diag blocks
            nc.vector.tensor_scalar(mask01[:], mask_t[:], -1.0, None,
                                    mybir.AluOpType.is_ge)
            vt0 = dma_v_input(0)
            nc.gpsimd.dma_start(
                wv_sb[:], wvT[:].rearrange("(i p) f -> p i f", p=P))
            nc.gpsimd.dma_start(
                wo_sb[:], woT[:].rearrange("(i p) d -> p i d", p=P))

            # chunk 0 projection inline; spin the PE through the k/v DMA
            # waits so the HAM clock-gate stays open
            qk0, v0 = proj_tasks(0, tiles0, vt0)
            for ti, task in enumerate(qk0 + v0):
                task()
                if ti == 5:
                    for wi in range(16):
                        nc.tensor.matmul(wps[:, wi % 2, 0:256], warm[:, 0:P],
                                         warm[:, 0:256], start=True, stop=True)
                elif ti == 11:
                    for wi in range(8):
                        nc.tensor.matmul(wps[:, wi % 2, 0:256], warm[:, 0:P],
                                         warm[:, 0:256], start=True, stop=True)

            deferred_v = None
            prefetched = {1: (dma_qk_inputs(1), dma_v_input(1))}
            for c in range(4):
                inner = deque()
                boundary = deque()
                if deferred_v is not None:
                    inner.extend(deferred_v)
                    deferred_v = None
                if c < 4 - 1:
                    nxt = c + 1
                    tiles, vt = prefetched.pop(nxt)
                    qk_t, v_t = proj_tasks(nxt, tiles, vt)
                    inner.extend(qk_t)
                    if nxt == 3:
                        deferred_v = v_t   # run inside attention(3) instead
                    else:
                        inner.extend(v_t)
                    if nxt + 1 <= 3:
                        def prefetch_task(cc=nxt + 1):
                            prefetched[cc] = (dma_qk_inputs(cc), dma_v_input(cc))
                        inner.insert(6, prefetch_task)
                for cc in {1: [0], 3: [1, 2]}.get(c, []):
                    for j in range(4):
                        boundary.append(lambda cc=cc, j=j: wo_task(cc, j))
                attention_chunk(c, inner, boundary, frontload=(c == 3))

            # final W_o for chunk 3 (psA slots are free now -> pipelined)
            for j in range(4):
                wo_task(3, j, pool=psA, ptag="b2")

    nc.compile()
    return nc


def _tables():
    inv = (1.0 / (ROPE_BASE ** (np.arange(0, Dh, 2, dtype=np.float32) / Dh))
           ).astype(np.float32)                      # [32]
    pos = np.arange(S, dtype=np.float32)
    ang = pos[:, None] * inv[None, :]                # [S, 32]
    cos = np.cos(ang).astype(np.float32)
    sin = np.sin(ang).astype(np.float32)
    d = np.arange(P) % Dh
    i = d // 2
    cosf = np.ascontiguousarray(cos[:, i].T)         # [128, S]
    sgn = np.where(d % 2 == 0, 1.0, -1.0).astype(np.float32)
    sinf = np.ascontiguousarray(sin[:, i].T * sgn[:, None]).astype(np.float32)

    p = np.arange(P)
    j = np.arange(P)
    maskA = np.where(p[:, None] <= j[None, :], 0.0, NEG).astype(np.float32)
    return cosf, sinf, maskA


def _core_inputs(q, k, v, W_q, W_k, W_v, W_o):
    """Build the 8 per-core input maps (bf16 transposed shards)."""
    cosf, sinf, maskA = _tables()
    in_maps = []
    for b in range(B):
        qTb = np.ascontiguousarray(q[b].T).astype(NPBF)
        kTb = np.ascontiguousarray(k[b].T).astype(NPBF)
        vTb = np.ascontiguousarray(v[b].T).astype(NPBF)
        for g in range(2):
            fs = slice(g * F, (g + 1) * F)
            in_maps.append({
                "qT": qTb, "kT": kTb, "vT": vTb,
                "wqT": np.ascontiguousarray(W_q[fs, :].T).astype(NPBF),
                "wkT": np.ascontiguousarray(W_k[fs, :].T).astype(NPBF),
                "wvT": np.ascontiguousarray(W_v[fs, :].T).astype(NPBF),
                "woT": np.ascontiguousarray(W_o[:, fs].T).astype(NPBF),
                "cosf": cosf, "sinf": sinf, "maskA": maskA,
            })
    return in_maps


def kernel(q, k, v, W_q, W_k, W_v, W_o):
    global _nc_cache, LAST_RESULT
    if _nc_cache is None:
        _nc_cache = _build_nc()
    nc = _nc_cache

    q = np.asarray(q, dtype=np.float32)
    k = np.asarray(k, dtype=np.float32)
    v = np.asarray(v, dtype=np.float32)
    W_q = np.asarray(W_q, dtype=np.float32)
    W_k = np.asarray(W_k, dtype=np.float32)
    W_v = np.asarray(W_v, dtype=np.float32)
    W_o = np.asarray(W_o, dtype=np.float32)

    in_maps = _core_inputs(q, k, v, W_q, W_k, W_v, W_o)

    res = bass_utils.run_bass_kernel_spmd(
        nc, in_maps, core_ids=list(range(N_CORES)), trace=KERNEL_TRACE)
    LAST_RESULT = res

    final = np.empty((B, S, D), dtype=np.float32)
    for b in range(B):
        final[b] = res.results[2 * b]["out"] + res.results[2 * b + 1]["out"]
    return final
